# revision 53
# baseline (speedup 1.0000x reference)
"""GraphSAGE (mean aggregation) on 8 Trainium2 NeuronCores.

v7 additions on top of v6 (default S_MODE="f8"): S matrices are pure
one-hots in fp8e4 (exact 1.0/0.0, so no precision loss feeding the f16 PE
matmuls) built in a single DVE is_equal pass; the mean 1/deg scaling moved
out of S into a fused DVE flush (PSUM f32 * invrow -> f16 SBUF) replacing
the ACT Identity flush; phase-B writeback chains are issued one window late;
aggregation windows are processed in PAIRS sharing one PSUM bank with a
single fused flush+scale per pair (halves PSUM->SBUF sem round-trips in
both phases). Measured HW total ~1.59-1.77 ms vs 1.82 ms baseline (axon
1x-vs-5x differencing, +-70 us noise; best observed 1.56 ms). Alternate modes kept for experiments:
"tt" (v6 two-pass f16 S), "ts" (per-block fused tensor_scalar — 4x slower
on HW), "dram" (host-built S streamed — slower), "pf8" (+KERNEL_PF8_NOSCOPE=1
KERNEL_GT=4: layer-invariant persistent S + 512-wide strip dense — neutral).

Strategy (v6):
  - Nodes partitioned across 8 cores (6250 real + pad -> 6400/core).
  - Full node-feature table h_all [51200, 128] fp16 replicated in each core's
    DRAM, laid out CHUNK-MAJOR: table row = h*25600 + m*3200 + (r - h*3200)
    for core m, local row r, chunk/half h = (r >= 3200). The per-layer
    AllGather is split into 2 contiguous chunk AllGathers, each issued as soon
    as the windows feeding it are written back -> chunk 0 of the next table
    overlaps the tail of the current layer, and the next layer's half-0
    gathers overlap chunk 1's AllGather.
  - Layers run in two phases: phase A aggregates half-0 edges for all windows
    (partial means flushed PSUM->SBUF f16 via the ACT engine), phase B adds
    half-1 edges; the dense SAGE transform accumulates Wl@partA + Wl@part2 +
    Wr@hT in PSUM, then relu (ACT), PE transpose to node-major, cc writeback.
  - Edge messages fetched with dma_gather (custom SWDGE gather, 4 queues,
    int16 indices, 1024-slot chunks spanning windows, slots sorted by src
    within each window for DRAM locality).
  - Mean aggregation = PE matmuls with one-hot S matrices batch-built on DVE
    per gather chunk; S carries the host-precomputed 1/deg(dst) scaling, so
    no per-window DVE combine work remains.
"""
import sys

sys.path.insert(0, "/opt/trn_rl_repo")

import numpy as np

import concourse.bass as bass
import concourse.bacc as bacc
import concourse.tile as tile
from concourse import mybir, library_config
from concourse.masks import make_identity

# problem constants (hardcoded per contract)
N, E, IN_DIM, HID, L = 50000, 625000, 300, 128, 4
NC = 8
NPC = N // NC            # 6250 real nodes per core
W_N = 128                # aggregation window width (psum free dim)
NW = 50                  # windows per core
NPAD = W_N * NW          # 6400 padded nodes per core
NTAB = NC * NPAD         # 51200 rows in the replicated table
HALF = NTAB // 2         # 25600 rows per table chunk (int16 idx limit)
RCH = NPAD // 2          # 3200 local rows per AllGather chunk
KCH = 3                  # 384 = 3*128 >= IN_DIM contraction chunks
import os as _os_mod
GMAX = int(_os_mod.environ.get("KERNEL_GMAX", "1024"))  # slots per dma_gather
DMA_SCRATCH = int(_os_mod.environ.get("KERNEL_DMA_SCRATCH", "16384"))
# S-build mode: "tt" = chunk-level tensor_tensor 2-pass (v6),
# "ts" = per-block fused tensor_scalar, "dram" = host-built, streamed
S_MODE = _os_mod.environ.get("KERNEL_S_MODE", "f8")
if _os_mod.environ.get("KERNEL_S_DRAM", "0") == "1":
    S_MODE = "dram"
S_DRAM = S_MODE == "dram"
PAG_BUFS = int(_os_mod.environ.get("KERNEL_PAG", "4"))
GT_BUFS = int(_os_mod.environ.get("KERNEL_GT", "6"))
NQ = int(_os_mod.environ.get("KERNEL_NQ", "4"))
PT_BUFS = int(_os_mod.environ.get("KERNEL_PT", "2"))

_CACHE = {}


def _host_prep(edge_index):
    """Build per-core gather streams, dst-offset blocks and program structure."""
    src = edge_index[0].astype(np.int64)
    dst = edge_index[1].astype(np.int64)
    # padded global ids
    gsrc = (src // NPC) * NPAD + (src % NPC)
    gdst = (dst // NPC) * NPAD + (dst % NPC)
    # chunk-major table index of each source
    m_s = gsrc // NPAD
    r_s = gsrc % NPAD
    h_s = (r_s >= RCH).astype(np.int64)
    tok_s = m_s * RCH + (r_s - h_s * RCH)      # 0..25599 within half

    per_core = []
    counts = np.zeros((NC, 2, NW), dtype=np.int64)
    for m in range(NC):
        sel = (gdst // NPAD) == m
        t_m = tok_s[sel]
        h_m = h_s[sel]
        dl = (gdst[sel] - m * NPAD).astype(np.int64)   # 0..6249
        w = dl // W_N
        # sort by (half, window, src-token) -> ascending DMA addresses
        order = np.lexsort((t_m, w, h_m))
        t_m, dl, h_m, w = t_m[order], dl[order], h_m[order], w[order]
        per_core.append((t_m, dl, h_m, w))
        for h in range(2):
            cw = np.bincount(w[h_m == h], minlength=NW)
            counts[m, h, :] = cw

    # uniform block structure across cores
    B = np.zeros((2, NW), dtype=np.int64)
    for h in range(2):
        for w in range(NW):
            B[h, w] = int(np.ceil(counts[:, h, w].max() / 128.0))

    slots_h = [int(B[h].sum() * 128) for h in range(2)]
    nb_h = [int(B[h].sum()) for h in range(2)]

    slot_off = np.zeros((2, NW), dtype=np.int64)
    for h in range(2):
        acc = 0
        for w in range(NW):
            slot_off[h, w] = acc
            acc += B[h, w] * 128

    # gather instruction chunks per half: (start_slot, n) spanning windows
    chunks = [[], []]
    for h in range(2):
        off = 0
        while off < slots_h[h]:
            n = min(GMAX, slots_h[h] - off)
            chunks[h].append((off, n))
            off += n

    idx_wrapped = []   # per core: [2][128, slots_h/16] int16
    dof_arr = []       # per core: [2][128, nb_h] fp32
    for m in range(NC):
        t_m, dl, h_m, w = per_core[m]
        iw_pair, dof_pair = [], []
        for h in range(2):
            tok = np.zeros(slots_h[h], dtype=np.int16)
            dof = np.full(slots_h[h], -1.0, dtype=np.float32)
            sel = h_m == h
            t_h, dl_h, w_h = t_m[sel], dl[sel], w[sel]
            for wi in range(NW):
                selw = w_h == wi
                cnt = int(selw.sum())
                if cnt == 0:
                    continue
                o = int(slot_off[h, wi])
                tok[o : o + cnt] = t_h[selw].astype(np.int16)
                dof[o : o + cnt] = (dl_h[selw] - wi * W_N).astype(np.float32)
            # wrap idx per gather instruction: j -> [j%16, j//16], replicated x8
            iw = np.zeros((128, slots_h[h] // 16), dtype=np.int16)
            for w0, n in chunks[h]:
                blockw = tok[w0 : w0 + n].reshape(n // 16, 16).T  # [16, n/16]
                iw[:, w0 // 16 : (w0 + n) // 16] = np.tile(blockw, (8, 1))
            iw_pair.append(iw)
            # dstoff partition-major: dof_arr[p, b] = dof[b*128+p]
            dof_pair.append(
                np.ascontiguousarray(dof.reshape(nb_h[h], 128).T).astype(np.float32)
            )
        idx_wrapped.append(iw_pair)
        dof_arr.append(dof_pair)

    # host-side inverse in-degree (pure edge_index preprocessing), folded
    # into per-slot scale values: invslot[p, b] = 1/deg(dst of slot b*128+p),
    # 0 for padding slots -> S matrices carry the mean scaling directly.
    deg = np.bincount(dst, minlength=N).astype(np.float32)
    inv = 1.0 / np.maximum(deg, 1.0)
    inv_pad = np.zeros((NC, NPAD), dtype=np.float32)
    inv_pad[:, :NPC] = inv.reshape(NC, NPC)

    invslot_arr = []  # per core: [2][128, nb_h] fp16
    sblk_arr = []     # per core: [2][128, nb_h, W_N] f16 host-built S blocks
    for m in range(NC):
        pair = []
        spair = []
        for h in range(2):
            dof = dof_arr[m][h].astype(np.float32)      # [128, nb]
            nb = dof.shape[1]
            # dst node of slot (p, b) = window(b)*W_N + dof
            wofb = np.zeros(nb, dtype=np.int64)
            for wi in range(NW):
                b0 = int(slot_off[h, wi]) // 128
                wofb[b0 : b0 + int(B[h, wi])] = wi
            dst_node = wofb[None, :] * W_N + dof.astype(np.int64)
            valid = dof >= 0
            iv = np.where(valid, inv_pad[m][np.clip(dst_node, 0, NPAD - 1)], 0.0)
            pair.append(iv.astype(np.float32))
            if S_DRAM:
                oh = (
                    dof[:, :, None] == np.arange(W_N, dtype=np.float32)[None, None, :]
                )
                spair.append(
                    (oh * iv[:, :, None]).astype(np.float16)
                )
        invslot_arr.append(pair)
        sblk_arr.append(spair)

    invrow_arr = [
        np.broadcast_to(inv_pad[m].astype(np.float16)[None, :], (128, NPAD)).copy()
        for m in range(NC)
    ]

    return {
        "sblk": sblk_arr,
        "invrow": invrow_arr,
        "B": B,
        "slots_h": slots_h,
        "nb_h": nb_h,
        "slot_off": slot_off,
        "chunks": chunks,
        "idx_wrapped": idx_wrapped,
        "dof": dof_arr,
        "invslot": invslot_arr,
    }


def _build_program(struct, timing_reps=1):
    B = struct["B"]
    slots_h = struct["slots_h"]
    nb_h = struct["nb_h"]
    slot_off = struct["slot_off"]
    chunks = struct["chunks"]

    nc = bacc.Bacc(
        "TRN2",
        target_bir_lowering=False,
        debug=False,
        num_devices=NC,
        num_swdge_queues=NQ,
        dynamic_dma_scratch_size=DMA_SCRATCH,
    )
    f32, f16, i16 = mybir.dt.float32, mybir.dt.float16, mybir.dt.int16
    f8 = mybir.dt.float8e4

    idx_d = [
        nc.dram_tensor(f"idx{h}", [128, max(slots_h[h] // 16, 1)], i16, kind="ExternalInput")
        for h in range(2)
    ]
    dof_d = [
        nc.dram_tensor(f"dof{h}", [128, max(nb_h[h], 1)], f32, kind="ExternalInput")
        for h in range(2)
    ]
    xT_d = nc.dram_tensor("xT", [KCH, 128, NPAD], f16, kind="ExternalInput")
    embW_d = nc.dram_tensor("embW", [KCH, 128, HID], f16, kind="ExternalInput")
    embB_d = nc.dram_tensor("embB", [128, 1], f32, kind="ExternalInput")
    Wl_d = nc.dram_tensor("Wl", [L, 128, HID], f16, kind="ExternalInput")
    Wr_d = nc.dram_tensor("Wr", [L, 128, HID], f16, kind="ExternalInput")
    bl_d = nc.dram_tensor("bl", [L, 128, 1], f32, kind="ExternalInput")
    iota_d = nc.dram_tensor("iota", [128, W_N], f16, kind="ExternalInput")
    invrow_d = nc.dram_tensor("invrow", [128, NPAD], f16, kind="ExternalInput")
    inval_d = [
        nc.dram_tensor(f"inval{h}", [128, max(nb_h[h], 1)], f32, kind="ExternalInput")
        for h in range(2)
    ]
    sblk_d = [
        nc.dram_tensor(f"sblk{h}", [128, max(nb_h[h], 1), W_N], f16, kind="ExternalInput")
        for h in range(2)
    ] if S_DRAM else None
    out_d = nc.dram_tensor("out", [NPAD, HID], f32, kind="ExternalOutput")

    rg = [list(range(NC))]
    qctr = [0]

    def next_q():
        q = qctr[0] % NQ
        qctr[0] += 1
        return q

    import os as _os
    _trace = _os.environ.get("KERNEL_TRACE_SIM") == "1"
    _ablate = _os.environ.get("KERNEL_ABLATE") == "1"
    _no_ag = _os.environ.get("KERNEL_NO_AG") == "1"
    _no_gather = _os.environ.get("KERNEL_NO_GATHER") == "1"
    _no_s = _os.environ.get("KERNEL_NO_S") == "1"
    with tile.TileContext(nc, trace_sim=_trace) as tc:
        with (
            tc.tile_pool(name="const", bufs=1) as constp,
            tc.tile_pool(name="big", bufs=1) as bigp,
            tc.tile_pool(name="gt", bufs=GT_BUFS) as gtp,
            tc.tile_pool(name="sp", bufs=GT_BUFS) as sp,
            tc.tile_pool(name="ap", bufs=4) as apool,
            tc.tile_pool(name="hp", bufs=4) as hpool,
            tc.tile_pool(name="pag", bufs=PAG_BUFS, space="PSUM") as pag,
            tc.tile_pool(name="pz", bufs=2, space="PSUM") as pz,
            tc.tile_pool(name="pt", bufs=PT_BUFS, space="PSUM") as pt,
            tc.tile_pool(name="dram", bufs=1, space="DRAM") as dram,
        ):
            nc.gpsimd.load_library(library_config.mlp)

            # --- resident constants / inputs in SBUF ---
            idx_sb = []
            dof_sb = []
            for h in range(2):
                t = constp.tile([128, max(slots_h[h] // 16, 1)], i16, name=f"idxsb{h}")
                nc.sync.dma_start(out=t[:], in_=idx_d[h][:])
                idx_sb.append(t)
                t2 = constp.tile([128, max(nb_h[h], 1)], f32, name=f"dofsb{h}")
                nc.sync.dma_start(out=t2[:], in_=dof_d[h][:])
                dof_sb.append(t2)
            inval_sb = []
            if S_MODE in ("tt", "ts"):
                for h in range(2):
                    t3 = constp.tile(
                        [128, max(nb_h[h], 1)], f32, name=f"invalsb{h}"
                    )
                    nc.sync.dma_start(out=t3[:], in_=inval_d[h][:])
                    inval_sb.append(t3)
            iota_sb = constp.tile([128, W_N], f16)
            nc.sync.dma_start(out=iota_sb[:], in_=iota_d[:])
            invrow_sb = constp.tile([128, NPAD], f16)
            nc.sync.dma_start(out=invrow_sb[:], in_=invrow_d[:])
            ident = constp.tile([128, 128], f32)
            make_identity(nc, ident[:])
            ident16 = constp.tile([128, 128], f16)
            nc.vector.tensor_copy(ident16[:], ident[:])
            embW_sb = constp.tile([128, KCH, HID], f16)
            nc.sync.dma_start(out=embW_sb[:], in_=embW_d[:].rearrange("k p h -> p k h"))
            embB_sb = constp.tile([128, 1], f32)
            nc.sync.dma_start(out=embB_sb[:], in_=embB_d[:])
            Wl_sb = constp.tile([128, L, HID], f16)
            nc.sync.dma_start(out=Wl_sb[:], in_=Wl_d[:].rearrange("l p h -> p l h"))
            Wr_sb = constp.tile([128, L, HID], f16)
            nc.sync.dma_start(out=Wr_sb[:], in_=Wr_d[:].rearrange("l p h -> p l h"))
            bl_sb = constp.tile([128, L], f32)
            nc.sync.dma_start(out=bl_sb[:], in_=bl_d[:].rearrange("l p one -> p (l one)"))
            hT = [bigp.tile([128, NPAD], f16, name=f"hT{i}") for i in range(2)]
            partA = bigp.tile([128, NPAD], f16, name="partA")  # phase-A means

            # DRAM buffers (fp16 table + per-layer AllGather outputs)
            n_ag = 1 + timing_reps * (L - 1)
            cc_in = [
                dram.tile([NPAD, HID], f16, name=f"ccin{i}", bufs=1) for i in range(2)
            ]
            h_all = [
                [
                    dram.tile(
                        [HALF, HID], f16, name=f"hall{i}_{k}", bufs=1,
                        addr_space="Shared",
                    )
                    for k in range(2)
                ]
                for i in range(n_ag)
            ]

            def writeback(hTbuf, w, dest):
                # transpose window back to node-major (fp16) and DMA to dest rows
                cs = slice(w * W_N, (w + 1) * W_N)
                ptile = pt.tile([128, 128], f16, tag="pt16", name="ptile")
                nc.tensor.transpose(ptile[:], hTbuf[:, cs], ident16[:])
                hsb = hpool.tile([128, 128], f16, tag="hsb", name="hsb")
                nc.scalar.activation(
                    hsb[:], ptile[:], mybir.ActivationFunctionType.Identity
                )
                nc.sync.dma_start(out=dest[cs, :], in_=hsb[:])

            def allgather(src_cc, dst_pair, k):
                ins_ap = src_cc[k * RCH : (k + 1) * RCH, :]
                if _no_ag:
                    nc.sync.dma_start(out=dst_pair[k][0:RCH, :], in_=ins_ap)
                    return
                nc.gpsimd.collective_compute(
                    "AllGather",
                    mybir.AluOpType.bypass,
                    replica_groups=rg,
                    ins=[ins_ap],
                    outs=[dst_pair[k][:]],
                )

            def embedding(xT_sb):
                for w in range(NW):
                    ws = slice(w * W_N, (w + 1) * W_N)
                    pzz = pz.tile([128, W_N], f32, tag="pz", name="pz")
                    for k in range(KCH):
                        nc.tensor.matmul(
                            pzz[:],
                            lhsT=embW_sb[:, k, :],
                            rhs=xT_sb[:, k, ws],
                            start=(k == 0),
                            stop=(k == KCH - 1),
                        )
                    nc.scalar.activation(
                        hT[0][:, ws], pzz[:], mybir.ActivationFunctionType.Relu,
                        bias=embB_sb[:],
                    )
                    writeback(hT[0], w, cc_in[0])
                    if w == NW // 2 - 1:
                        allgather(cc_in[0], h_all[0], 0)
                allgather(cc_in[0], h_all[0], 1)

            def layer(l, h_src, hT_in, hT_out, dest, next_hall, S_all=None):
                half_ap = [h_src[0][:], h_src[1][:]]
                issued = [{}, {}]
                nexti = [0, 0]

                def ensure_chunk(h, ci):
                    while nexti[h] <= ci:
                        w0, n = chunks[h][nexti[h]]
                        nb = n // 128
                        sdt = f8 if S_MODE == "f8" else f16
                        gt = gtp.tile([128, nb, 128], f16, tag="gt", name="gt")
                        if _no_gather:
                            nc.vector.memset(gt[:], 0.0)
                        else:
                            nc.gpsimd.dma_gather(
                                gt[:],
                                half_ap[h],
                                idx_sb[h][:, w0 // 16 : (w0 + n) // 16],
                                n,
                                n,
                                HID,
                                queue_num=next_q(),
                            )
                        if S_MODE == "pf8":
                            # persistent layer-invariant S; no per-chunk build
                            issued[h][nexti[h]] = (gt, None)
                            nexti[h] += 1
                            continue
                        # batched one-hot S for all nb blocks of this chunk,
                        # scaled per-slot by 1/deg(dst) (mean aggregation)
                        b0 = w0 // 128
                        S = sp.tile([128, nb, W_N], sdt, tag="S", name="S")
                        if _no_s:
                            nc.vector.memset(S[:], 0.0)
                        elif S_MODE == "f8":
                            # pure one-hot in fp8 (exact); 1/deg applied to the
                            # flushed partials instead -> single DVE pass.
                            nc.vector.tensor_tensor(
                                out=S[:],
                                in0=dof_sb[h][:, b0 : b0 + nb, None].broadcast_to(
                                    [128, nb, W_N]
                                ),
                                in1=iota_sb[:, None, :].broadcast_to([128, nb, W_N]),
                                op=mybir.AluOpType.is_equal,
                            )
                        elif S_MODE == "dram":
                            # S blocks precomputed on host (pure edge_index
                            # preprocessing); stream from DRAM, no DVE work.
                            nc.sync.dma_start(
                                out=S[:], in_=sblk_d[h][:, b0 : b0 + nb, :]
                            )
                        elif S_MODE == "ts":
                            # fused one-hot build: S[:,j,:] = (iota == dof_j) * inval_j
                            # per-partition scalars keep all tensor operands
                            # packed f16/SBUF -> DVE 4x_2p fast mode.
                            for j in range(nb):
                                nc.vector.tensor_scalar(
                                    out=S[:, j, :],
                                    in0=iota_sb[:],
                                    scalar1=dof_sb[h][:, b0 + j : b0 + j + 1],
                                    scalar2=inval_sb[h][:, b0 + j : b0 + j + 1],
                                    op0=mybir.AluOpType.is_equal,
                                    op1=mybir.AluOpType.mult,
                                )
                        else:
                            nc.vector.tensor_tensor(
                                out=S[:],
                                in0=dof_sb[h][:, b0 : b0 + nb, None].broadcast_to(
                                    [128, nb, W_N]
                                ),
                                in1=iota_sb[:, None, :].broadcast_to([128, nb, W_N]),
                                op=mybir.AluOpType.is_equal,
                            )
                            nc.vector.tensor_tensor(
                                out=S[:],
                                in0=S[:],
                                in1=inval_sb[h][:, b0 : b0 + nb, None].broadcast_to(
                                    [128, nb, W_N]
                                ),
                                op=mybir.AluOpType.mult,
                            )
                        issued[h][nexti[h]] = (gt, S)
                        nexti[h] += 1

                def agg_blocks(h, w, pagg):
                    nb = int(B[h][w])
                    for b in range(nb):
                        ab = int(slot_off[h][w]) + b * 128
                        ci = ab // GMAX
                        ensure_chunk(h, ci)
                        if _ablate:
                            continue
                        gt, S = issued[h][ci]
                        j = (ab % GMAX) // 128
                        rhs = (
                            S_all[h][:, ab // 128, :]
                            if S_MODE == "pf8"
                            else S[:, j, :]
                        )
                        nc.tensor.matmul(
                            pagg[:],
                            lhsT=gt[:, j, :],
                            rhs=rhs,
                            start=(b == 0),
                            stop=(b == nb - 1),
                        )
                    return nb > 0 and not _ablate

                # ---- phase A: half-0 partial means for all windows ----
                if S_MODE in ("f8", "pf8"):
                    # windows processed in pairs sharing one PSUM bank with a
                    # single fused flush+scale per pair: halves the number of
                    # PSUM->SBUF round-trips (sem chains) in phase A.
                    for wp in range(0, NW, 4):
                        gsz = min(4, NW - wp)
                        pagg2 = pag.tile(
                            [128, 4, W_N], f32, tag="pagg", name="pagg"
                        )
                        got = [
                            agg_blocks(0, wp + k, pagg2[:, k, :])
                            for k in range(gsz)
                        ]
                        ws2 = slice(wp * W_N, (wp + gsz) * W_N)
                        if all(got):
                            nc.vector.tensor_tensor(
                                out=partA[:, ws2],
                                in0=pagg2[:, 0:gsz, :].rearrange(
                                    "p a b -> p (a b)"
                                ),
                                in1=invrow_sb[:, ws2],
                                op=mybir.AluOpType.mult,
                            )
                        else:
                            for k in range(gsz):
                                wk = slice(
                                    (wp + k) * W_N, (wp + k + 1) * W_N
                                )
                                if got[k]:
                                    nc.vector.tensor_tensor(
                                        out=partA[:, wk],
                                        in0=pagg2[:, k, :],
                                        in1=invrow_sb[:, wk],
                                        op=mybir.AluOpType.mult,
                                    )
                                else:
                                    nc.vector.memset(partA[:, wk], 0.0)
                else:
                    for w in range(NW):
                        ws = slice(w * W_N, (w + 1) * W_N)
                        pagg = pag.tile([128, W_N], f32, tag="paggs", name="pagg")
                        if agg_blocks(0, w, pagg):
                            nc.scalar.activation(
                                partA[:, ws], pagg[:],
                                mybir.ActivationFunctionType.Identity,
                            )
                        else:
                            nc.vector.memset(partA[:, ws], 0.0)

                # ---- phase B: half-1 + dense + writeback ----
                # the writeback chain (PE transpose -> ACT/DVE copy -> DMA) of
                # window w is issued one window late so the in-order PE queue
                # never stalls on window w's relu before starting w+1's aggs.
                pending = []

                def flush_pending():
                    while pending:
                        pending.pop(0)()

                if S_MODE == "pf8":
                    # strip-dense: phase-B partials are ADDED into partA on the
                    # (idle) DVE, then the dense transform runs in 512-wide
                    # strips: 2 matmuls + 1 relu per strip instead of 3+1 per
                    # 128-window -> ~250 fewer PE insts and 37 fewer ACT insts
                    # per layer.
                    SW = 512
                    for s0 in range(0, NPAD, SW):
                        sw = min(SW, NPAD - s0)
                        for w in range(s0 // W_N, (s0 + sw) // W_N):
                            ws = slice(w * W_N, (w + 1) * W_N)
                            pagg = pag.tile([128, W_N], f32, tag="paggs", name="pagg")
                            if agg_blocks(1, w, pagg):
                                tmp = apool.tile([128, W_N], f16, tag="p2", name="p2")
                                nc.vector.tensor_tensor(
                                    out=tmp[:], in0=pagg[:],
                                    in1=invrow_sb[:, ws], op=mybir.AluOpType.mult,
                                )
                                nc.vector.tensor_tensor(
                                    out=partA[:, ws], in0=partA[:, ws],
                                    in1=tmp[:], op=mybir.AluOpType.add,
                                )
                        while len(pending) > 1:
                            pending.pop(0)()
                        ss = slice(s0, s0 + sw)
                        pzz = pz.tile([128, SW], f32, tag="pz", name="pz")
                        nc.tensor.matmul(
                            pzz[:, :sw], lhsT=Wl_sb[:, l, :], rhs=partA[:, ss],
                            start=True, stop=False,
                        )
                        nc.tensor.matmul(
                            pzz[:, :sw], lhsT=Wr_sb[:, l, :], rhs=hT_in[:, ss],
                            start=False, stop=True,
                        )
                        if l < L - 1:
                            nc.scalar.activation(
                                hT_out[:, ss], pzz[:, :sw],
                                mybir.ActivationFunctionType.Relu,
                                bias=bl_sb[:, l : l + 1],
                            )

                            def wb(s0=s0, sw=sw):
                                for w in range(s0 // W_N, (s0 + sw) // W_N):
                                    writeback(hT_out, w, dest)
                                    if w == NW // 2 - 1:
                                        allgather(dest, next_hall, 0)
                                    elif w == NW - 1:
                                        allgather(dest, next_hall, 1)

                            pending.append(wb)
                        else:
                            h4s = apool.tile([128, SW], f16, tag="h4s", name="h4s")
                            nc.scalar.activation(
                                h4s[:, :sw], pzz[:, :sw],
                                mybir.ActivationFunctionType.Relu,
                                bias=bl_sb[:, l : l + 1],
                            )

                            def wb(s0=s0, sw=sw, h4s=h4s):
                                for wi, w in enumerate(
                                    range(s0 // W_N, (s0 + sw) // W_N)
                                ):
                                    cs = slice(w * W_N, (w + 1) * W_N)
                                    ptile = pt.tile(
                                        [128, 128], f16, tag="pt16", name="ptile"
                                    )
                                    nc.tensor.transpose(
                                        ptile[:],
                                        h4s[:, wi * 128 : (wi + 1) * 128],
                                        ident16[:],
                                    )
                                    hsb = hpool.tile(
                                        [128, 128], f32, tag="hsbo", name="hsbo"
                                    )
                                    nc.vector.tensor_copy(hsb[:], ptile[:])
                                    nc.sync.dma_start(out=out_d[cs, :], in_=hsb[:])

                            pending.append(wb)
                    flush_pending()
                    return

                def dense_relu_wb(w, got, part2_ap):
                    ws = slice(w * W_N, (w + 1) * W_N)
                    pzz = pz.tile([128, W_N], f32, tag="pz", name="pz")
                    nc.tensor.matmul(
                        pzz[:], lhsT=Wl_sb[:, l, :], rhs=partA[:, ws],
                        start=True, stop=False,
                    )
                    if got:
                        nc.tensor.matmul(
                            pzz[:], lhsT=Wl_sb[:, l, :], rhs=part2_ap,
                            start=False, stop=False,
                        )
                    nc.tensor.matmul(
                        pzz[:], lhsT=Wr_sb[:, l, :], rhs=hT_in[:, ws], start=False,
                        stop=True,
                    )
                    if l < L - 1:
                        nc.scalar.activation(
                            hT_out[:, ws], pzz[:],
                            mybir.ActivationFunctionType.Relu,
                            bias=bl_sb[:, l : l + 1],
                        )

                        def wb(w=w):
                            writeback(hT_out, w, dest)
                            if w == NW // 2 - 1:
                                allgather(dest, next_hall, 0)
                            elif w == NW - 1:
                                allgather(dest, next_hall, 1)

                        pending.append(wb)
                    else:
                        h4 = apool.tile([128, W_N], f16, tag="h4", name="h4")
                        nc.scalar.activation(
                            h4[:], pzz[:],
                            mybir.ActivationFunctionType.Relu,
                            bias=bl_sb[:, l : l + 1],
                        )

                        def wb(w=w, h4=h4):
                            cs = slice(w * W_N, (w + 1) * W_N)
                            ptile = pt.tile([128, 128], f16, tag="pt16", name="ptile")
                            nc.tensor.transpose(ptile[:], h4[:], ident16[:])
                            hsb = hpool.tile([128, 128], f32, tag="hsbo", name="hsbo")
                            nc.vector.tensor_copy(hsb[:], ptile[:])
                            nc.sync.dma_start(out=out_d[cs, :], in_=hsb[:])

                        pending.append(wb)
                    if len(pending) > 1:
                        pending.pop(0)()

                if S_MODE in ("f8", "pf8"):
                    # paired phase-B: two windows share one PSUM bank and one
                    # fused flush+scale, then each window's dense runs.
                    for wp in range(0, NW, 4):
                        gsz = min(4, NW - wp)
                        pagg2b = pag.tile(
                            [128, 4, W_N], f32, tag="pagg", name="pagg"
                        )
                        gots = [
                            agg_blocks(1, wp + k, pagg2b[:, k, :])
                            for k in range(gsz)
                        ]
                        part2 = apool.tile(
                            [128, 4, W_N], f16, tag="p2", name="p2"
                        )
                        ws2 = slice(wp * W_N, (wp + gsz) * W_N)
                        if all(gots):
                            nc.vector.tensor_tensor(
                                out=part2[:, 0:gsz, :].rearrange(
                                    "p a b -> p (a b)"
                                ),
                                in0=pagg2b[:, 0:gsz, :].rearrange(
                                    "p a b -> p (a b)"
                                ),
                                in1=invrow_sb[:, ws2],
                                op=mybir.AluOpType.mult,
                            )
                        else:
                            for k in range(gsz):
                                if gots[k]:
                                    wk = slice(
                                        (wp + k) * W_N, (wp + k + 1) * W_N
                                    )
                                    nc.vector.tensor_tensor(
                                        out=part2[:, k, :],
                                        in0=pagg2b[:, k, :],
                                        in1=invrow_sb[:, wk],
                                        op=mybir.AluOpType.mult,
                                    )
                        for k in range(gsz):
                            dense_relu_wb(wp + k, gots[k], part2[:, k, :])
                    flush_pending()
                    return

                for w in range(NW):
                    ws = slice(w * W_N, (w + 1) * W_N)
                    pagg = pag.tile([128, W_N], f32, tag="paggs", name="pagg")
                    got = agg_blocks(1, w, pagg)
                    if got:
                        part2 = apool.tile([128, W_N], f16, tag="p2s", name="p2")
                        nc.scalar.activation(
                            part2[:], pagg[:],
                            mybir.ActivationFunctionType.Identity,
                        )
                    pzz = pz.tile([128, W_N], f32, tag="pz", name="pz")
                    nc.tensor.matmul(
                        pzz[:], lhsT=Wl_sb[:, l, :], rhs=partA[:, ws],
                        start=True, stop=False,
                    )
                    if got:
                        nc.tensor.matmul(
                            pzz[:], lhsT=Wl_sb[:, l, :], rhs=part2[:],
                            start=False, stop=False,
                        )
                    nc.tensor.matmul(
                        pzz[:], lhsT=Wr_sb[:, l, :], rhs=hT_in[:, ws], start=False,
                        stop=True,
                    )
                    if l < L - 1:
                        nc.scalar.activation(
                            hT_out[:, ws], pzz[:],
                            mybir.ActivationFunctionType.Relu,
                            bias=bl_sb[:, l : l + 1],
                        )

                        def wb(w=w):
                            writeback(hT_out, w, dest)
                            if w == NW // 2 - 1:
                                allgather(dest, next_hall, 0)
                            elif w == NW - 1:
                                allgather(dest, next_hall, 1)

                        pending.append(wb)
                    else:
                        h4 = apool.tile([128, W_N], f16, tag="h4", name="h4")
                        nc.scalar.activation(
                            h4[:], pzz[:],
                            mybir.ActivationFunctionType.Relu,
                            bias=bl_sb[:, l : l + 1],
                        )

                        def wb(w=w, h4=h4):
                            cs = slice(w * W_N, (w + 1) * W_N)
                            ptile = pt.tile([128, 128], f16, tag="pt16", name="ptile")
                            nc.tensor.transpose(ptile[:], h4[:], ident16[:])
                            hsb = hpool.tile([128, 128], f32, tag="hsbo", name="hsbo")
                            nc.vector.tensor_copy(hsb[:], ptile[:])
                            nc.sync.dma_start(out=out_d[cs, :], in_=hsb[:])

                        pending.append(wb)
                    if len(pending) > 1:
                        pending.pop(0)()
                flush_pending()

            import os as _os2
            _noscope = _os2.environ.get("KERNEL_PF8_NOSCOPE", "0") == "1"
            if _noscope:
                xT_sb = bigp.tile([128, KCH, NPAD], f16, name="xT")
                nc.sync.dma_start(
                    out=xT_sb[:], in_=xT_d[:].rearrange("k p n -> p k n")
                )
                embedding(xT_sb)
            else:
                with tc.tile_pool(name="xt", bufs=1) as xtp:
                    xT_sb = xtp.tile([128, KCH, NPAD], f16)
                    nc.sync.dma_start(
                        out=xT_sb[:], in_=xT_d[:].rearrange("k p n -> p k n")
                    )
                    embedding(xT_sb)

            def run_layers(S_all=None):
                agi = 0
                for rep in range(timing_reps):
                    for l in range(L):
                        layer(
                            l,
                            h_all[agi],
                            hT[l % 2],
                            hT[(l + 1) % 2],
                            cc_in[(l + 1) % 2],
                            h_all[agi + 1] if l < L - 1 else None,
                            S_all=S_all,
                        )
                        if l < L - 1:
                            agi += 1

            if S_MODE == "pf8":
                # layer-invariant one-hot S built once in fp8, resident in the
                # SBUF region vacated by the embedding's xT tile
                with tc.tile_pool(name="sall", bufs=1) as sallp:
                    S_all = [
                        sallp.tile(
                            [128, max(nb_h[h], 1), W_N], f8, name=f"sall{h}"
                        )
                        for h in range(2)
                    ]
                    for h in range(2):
                        for w0, n in chunks[h]:
                            nb = n // 128
                            b0 = w0 // 128
                            nc.vector.tensor_tensor(
                                out=S_all[h][:, b0 : b0 + nb, :],
                                in0=dof_sb[h][:, b0 : b0 + nb, None].broadcast_to(
                                    [128, nb, W_N]
                                ),
                                in1=iota_sb[:, None, :].broadcast_to(
                                    [128, nb, W_N]
                                ),
                                op=mybir.AluOpType.is_equal,
                            )
                    run_layers(S_all)
            else:
                run_layers()

    nc.compile()
    return nc


def _prep_inputs(inputs, struct):
    x = np.asarray(inputs["x"], dtype=np.float32)
    emb_W = np.asarray(inputs["emb_W"], dtype=np.float32)
    emb_b = np.asarray(inputs["emb_b"], dtype=np.float32)
    Wl = np.asarray(inputs["Wl"], dtype=np.float32)
    bl = np.asarray(inputs["bl"], dtype=np.float32)
    Wr = np.asarray(inputs["Wr"], dtype=np.float32)

    embW_p = np.zeros((KCH, 128, HID), dtype=np.float16)
    embW_p.reshape(KCH * 128, HID)[:IN_DIM] = emb_W.astype(np.float16)
    embB_p = np.zeros((128, 1), dtype=np.float32)
    embB_p[:, 0] = emb_b
    Wl_p = Wl.astype(np.float16)
    Wr_p = Wr.astype(np.float16)
    bl_p = np.ascontiguousarray(bl[:, :, None].astype(np.float32))

    iota = np.broadcast_to(
        np.arange(W_N, dtype=np.float16)[None, :], (128, W_N)
    ).copy()

    in_maps = []
    for m in range(NC):
        xm = np.zeros((KCH * 128, NPAD), dtype=np.float16)
        xm[:IN_DIM, :NPC] = x[m * NPC : (m + 1) * NPC].T.astype(np.float16)
        im = {
            "idx0": struct["idx_wrapped"][m][0],
            "idx1": struct["idx_wrapped"][m][1],
            "dof0": struct["dof"][m][0],
            "dof1": struct["dof"][m][1],
            "inval0": struct["invslot"][m][0],
            "inval1": struct["invslot"][m][1],
            **(
                {"sblk0": struct["sblk"][m][0], "sblk1": struct["sblk"][m][1]}
                if S_DRAM
                else {}
            ),
            "xT": xm.reshape(KCH, 128, NPAD),
            "embW": embW_p,
            "embB": embB_p,
            "Wl": Wl_p,
            "Wr": Wr_p,
            "bl": bl_p,
            "iota": iota,
            "invrow": struct["invrow"][m],
        }
        in_maps.append(im)
    return in_maps


class BassRunner:
    """Executes a compiled Bass program via PJRT/axon; jit built once."""

    def __init__(self, nc, n_cores):
        import jax
        from jax.sharding import Mesh, PartitionSpec
        from jax.experimental.shard_map import shard_map
        from concourse.bass2jax import (
            _bass_exec_p,
            install_neuronx_cc_hook,
            partition_id_tensor,
        )

        install_neuronx_cc_hook()
        self.jax = jax
        self.nc = nc
        self.n_cores = n_cores
        partition_name = (
            nc.partition_id_tensor.name if nc.partition_id_tensor else None
        )
        in_names, out_names, out_avals, zero_outs = [], [], [], []
        for alloc in nc.m.functions[0].allocations:
            if not isinstance(alloc, mybir.MemoryLocationSet):
                continue
            name = alloc.memorylocations[0].name
            if alloc.kind == "ExternalInput":
                if name != partition_name:
                    in_names.append(name)
            elif alloc.kind == "ExternalOutput":
                shape = tuple(alloc.tensor_shape)
                dtype = mybir.dt.np(alloc.dtype)
                out_names.append(name)
                out_avals.append(jax.core.ShapedArray(shape, dtype))
                zero_outs.append(np.zeros(shape, dtype))
        self.in_names, self.out_names = in_names, out_names
        self.zero_outs, self._out_avals = zero_outs, out_avals
        n_params, n_outs = len(in_names), len(out_avals)
        all_in_names = in_names + out_names
        if partition_name is not None:
            all_in_names = all_in_names + [partition_name]

        def _body(*args):
            operands = list(args)
            if partition_name is not None:
                operands.append(partition_id_tensor())
            return tuple(
                _bass_exec_p.bind(
                    *operands,
                    out_avals=tuple(out_avals),
                    in_names=tuple(all_in_names),
                    out_names=tuple(out_names),
                    lowering_input_output_aliases=(),
                    sim_require_finite=True,
                    sim_require_nnan=True,
                    nc=nc,
                )
            )

        devices = jax.devices()[:n_cores]
        self._mesh = Mesh(np.asarray(devices), ("core",))
        self._pspec = PartitionSpec("core")
        in_specs = (self._pspec,) * (n_params + n_outs)
        out_specs = (self._pspec,) * len(out_names)
        self._fn = jax.jit(
            shard_map(
                _body,
                mesh=self._mesh,
                in_specs=in_specs,
                out_specs=out_specs,
                check_rep=False,
            ),
            keep_unused=True,
        )

    def prepare(self, in_maps):
        n = self.n_cores
        concat_in = [
            np.concatenate(
                [np.asarray(in_maps[c][name]) for c in range(n)], axis=0
            )
            for name in self.in_names
        ]
        concat_zeros = [
            np.zeros((n * z.shape[0], *z.shape[1:]), z.dtype)
            for z in self.zero_outs
        ]
        sharding = self.jax.sharding.NamedSharding(self._mesh, self._pspec)
        self._args = [
            self.jax.device_put(a, sharding) for a in concat_in + concat_zeros
        ]

    def execute(self):
        outs = self._fn(*self._args)
        self.jax.block_until_ready(outs)
        return outs

    def run(self):
        outs = self.execute()
        n = self.n_cores
        return [
            {
                name: np.asarray(outs[i]).reshape(
                    n, *self._out_avals[i].shape
                )[c]
                for i, name in enumerate(self.out_names)
            }
            for c in range(n)
        ]


def _get_runner(edge_index, timing_reps=1):
    import os as _os
    _flags = (
        _os.environ.get("KERNEL_NO_AG", ""),
        _os.environ.get("KERNEL_ABLATE", ""),
        _os.environ.get("KERNEL_NO_GATHER", ""),
        _os.environ.get("KERNEL_NO_S", ""),
        GMAX,
        DMA_SCRATCH,
        S_MODE,
        PAG_BUFS,
        GT_BUFS,
        NQ,
        PT_BUFS,
    )
    key = ("prog", timing_reps, _flags, hash(edge_index.tobytes()))
    if key in _CACHE:
        return _CACHE[key]
    struct = _host_prep(edge_index)
    nc = _build_program(struct, timing_reps=timing_reps)
    runner = BassRunner(nc, NC)
    _CACHE[key] = (struct, runner)
    return struct, runner


def kernel(**inputs):
    edge_index = np.asarray(inputs["edge_index"])
    struct, runner = _get_runner(edge_index)
    in_maps = _prep_inputs(inputs, struct)
    runner.prepare(in_maps)
    results = runner.run()
    out = np.empty((N, HID), dtype=np.float32)
    for m in range(NC):
        out[m * NPC : (m + 1) * NPC] = results[m]["out"][:NPC]
    return out



# revision 54
# speedup vs baseline: 1.0461x; 1.0461x over previous
"""GraphSAGE (mean aggregation) on 8 Trainium2 NeuronCores.

v7 additions on top of v6 (default S_MODE="f8"): S matrices are pure
one-hots in fp8e4 (exact 1.0/0.0, so no precision loss feeding the f16 PE
matmuls) built in a single DVE is_equal pass; the mean 1/deg scaling moved
out of S into a fused DVE flush (PSUM f32 * invrow -> f16 SBUF) replacing
the ACT Identity flush; phase-B writeback chains are issued one window late;
aggregation windows are processed in PAIRS sharing one PSUM bank with a
single fused flush+scale per pair (halves PSUM->SBUF sem round-trips in
both phases). Measured HW total ~1.59-1.77 ms vs 1.82 ms baseline (axon
1x-vs-5x differencing, +-70 us noise; best observed 1.56 ms). Alternate modes kept for experiments:
"tt" (v6 two-pass f16 S), "ts" (per-block fused tensor_scalar — 4x slower
on HW), "dram" (host-built S streamed — slower), "pf8" (+KERNEL_PF8_NOSCOPE=1
KERNEL_GT=4: layer-invariant persistent S + 512-wide strip dense — neutral).

Strategy (v6):
  - Nodes partitioned across 8 cores (6250 real + pad -> 6400/core).
  - Full node-feature table h_all [51200, 128] fp16 replicated in each core's
    DRAM, laid out CHUNK-MAJOR: table row = h*25600 + m*3200 + (r - h*3200)
    for core m, local row r, chunk/half h = (r >= 3200). The per-layer
    AllGather is split into 2 contiguous chunk AllGathers, each issued as soon
    as the windows feeding it are written back -> chunk 0 of the next table
    overlaps the tail of the current layer, and the next layer's half-0
    gathers overlap chunk 1's AllGather.
  - Layers run in two phases: phase A aggregates half-0 edges for all windows
    (partial means flushed PSUM->SBUF f16 via the ACT engine), phase B adds
    half-1 edges; the dense SAGE transform accumulates Wl@partA + Wl@part2 +
    Wr@hT in PSUM, then relu (ACT), PE transpose to node-major, cc writeback.
  - Edge messages fetched with dma_gather (custom SWDGE gather, 4 queues,
    int16 indices, 1024-slot chunks spanning windows, slots sorted by src
    within each window for DRAM locality).
  - Mean aggregation = PE matmuls with one-hot S matrices batch-built on DVE
    per gather chunk; S carries the host-precomputed 1/deg(dst) scaling, so
    no per-window DVE combine work remains.
"""
import sys

sys.path.insert(0, "/opt/trn_rl_repo")

import numpy as np

import concourse.bass as bass
import concourse.bacc as bacc
import concourse.tile as tile
from concourse import mybir, library_config
from concourse.masks import make_identity

# problem constants (hardcoded per contract)
N, E, IN_DIM, HID, L = 50000, 625000, 300, 128, 4
NC = 8
NPC = N // NC            # 6250 real nodes per core
W_N = 128                # aggregation window width (psum free dim)
NW = 50                  # windows per core
NPAD = W_N * NW          # 6400 padded nodes per core
NTAB = NC * NPAD         # 51200 rows in the replicated table
HALF = NTAB // 2         # 25600 rows per table chunk (int16 idx limit)
RCH = NPAD // 2          # 3200 local rows per AllGather chunk
KCH = 3                  # 384 = 3*128 >= IN_DIM contraction chunks
import os as _os_mod
GMAX = int(_os_mod.environ.get("KERNEL_GMAX", "1024"))  # slots per dma_gather
DMA_SCRATCH = int(_os_mod.environ.get("KERNEL_DMA_SCRATCH", "16384"))
# S-build mode: "tt" = chunk-level tensor_tensor 2-pass (v6),
# "ts" = per-block fused tensor_scalar, "dram" = host-built, streamed
S_MODE = _os_mod.environ.get("KERNEL_S_MODE", "f8")
if _os_mod.environ.get("KERNEL_S_DRAM", "0") == "1":
    S_MODE = "dram"
S_DRAM = S_MODE == "dram"
PAG_BUFS = int(_os_mod.environ.get("KERNEL_PAG", "4"))
GT_BUFS = int(_os_mod.environ.get("KERNEL_GT", "6"))
NQ = int(_os_mod.environ.get("KERNEL_NQ", "4"))
PT_BUFS = int(_os_mod.environ.get("KERNEL_PT", "2"))

_CACHE = {}


def _host_prep(edge_index):
    """Build per-core gather streams, dst-offset blocks and program structure."""
    src = edge_index[0].astype(np.int64)
    dst = edge_index[1].astype(np.int64)
    # padded global ids
    gsrc = (src // NPC) * NPAD + (src % NPC)
    gdst = (dst // NPC) * NPAD + (dst % NPC)
    # chunk-major table index of each source
    m_s = gsrc // NPAD
    r_s = gsrc % NPAD
    h_s = (r_s >= RCH).astype(np.int64)
    tok_s = m_s * RCH + (r_s - h_s * RCH)      # 0..25599 within half

    per_core = []
    counts = np.zeros((NC, 2, NW), dtype=np.int64)
    for m in range(NC):
        sel = (gdst // NPAD) == m
        t_m = tok_s[sel]
        h_m = h_s[sel]
        dl = (gdst[sel] - m * NPAD).astype(np.int64)   # 0..6249
        w = dl // W_N
        # sort by (half, window, src-token) -> ascending DMA addresses
        order = np.lexsort((t_m, w, h_m))
        t_m, dl, h_m, w = t_m[order], dl[order], h_m[order], w[order]
        per_core.append((t_m, dl, h_m, w))
        for h in range(2):
            cw = np.bincount(w[h_m == h], minlength=NW)
            counts[m, h, :] = cw

    # uniform block structure across cores
    B = np.zeros((2, NW), dtype=np.int64)
    for h in range(2):
        for w in range(NW):
            B[h, w] = int(np.ceil(counts[:, h, w].max() / 128.0))

    slots_h = [int(B[h].sum() * 128) for h in range(2)]
    nb_h = [int(B[h].sum()) for h in range(2)]

    slot_off = np.zeros((2, NW), dtype=np.int64)
    for h in range(2):
        acc = 0
        for w in range(NW):
            slot_off[h, w] = acc
            acc += B[h, w] * 128

    # gather instruction chunks per half: (start_slot, n) spanning windows
    chunks = [[], []]
    for h in range(2):
        off = 0
        while off < slots_h[h]:
            n = min(GMAX, slots_h[h] - off)
            chunks[h].append((off, n))
            off += n

    idx_wrapped = []   # per core: [2][128, slots_h/16] int16
    dof_arr = []       # per core: [2][128, nb_h] fp32
    for m in range(NC):
        t_m, dl, h_m, w = per_core[m]
        iw_pair, dof_pair = [], []
        for h in range(2):
            tok = np.zeros(slots_h[h], dtype=np.int16)
            dof = np.full(slots_h[h], -1.0, dtype=np.float32)
            sel = h_m == h
            t_h, dl_h, w_h = t_m[sel], dl[sel], w[sel]
            for wi in range(NW):
                selw = w_h == wi
                cnt = int(selw.sum())
                if cnt == 0:
                    continue
                o = int(slot_off[h, wi])
                tok[o : o + cnt] = t_h[selw].astype(np.int16)
                dof[o : o + cnt] = (dl_h[selw] - wi * W_N).astype(np.float32)
            # wrap idx per gather instruction: j -> [j%16, j//16], replicated x8
            iw = np.zeros((128, slots_h[h] // 16), dtype=np.int16)
            for w0, n in chunks[h]:
                blockw = tok[w0 : w0 + n].reshape(n // 16, 16).T  # [16, n/16]
                iw[:, w0 // 16 : (w0 + n) // 16] = np.tile(blockw, (8, 1))
            iw_pair.append(iw)
            # dstoff partition-major: dof_arr[p, b] = dof[b*128+p]
            dof_pair.append(
                np.ascontiguousarray(dof.reshape(nb_h[h], 128).T).astype(np.float32)
            )
        idx_wrapped.append(iw_pair)
        dof_arr.append(dof_pair)

    # host-side inverse in-degree (pure edge_index preprocessing), folded
    # into per-slot scale values: invslot[p, b] = 1/deg(dst of slot b*128+p),
    # 0 for padding slots -> S matrices carry the mean scaling directly.
    deg = np.bincount(dst, minlength=N).astype(np.float32)
    inv = 1.0 / np.maximum(deg, 1.0)
    inv_pad = np.zeros((NC, NPAD), dtype=np.float32)
    inv_pad[:, :NPC] = inv.reshape(NC, NPC)

    invslot_arr = []  # per core: [2][128, nb_h] fp16
    sblk_arr = []     # per core: [2][128, nb_h, W_N] f16 host-built S blocks
    for m in range(NC):
        pair = []
        spair = []
        for h in range(2):
            dof = dof_arr[m][h].astype(np.float32)      # [128, nb]
            nb = dof.shape[1]
            # dst node of slot (p, b) = window(b)*W_N + dof
            wofb = np.zeros(nb, dtype=np.int64)
            for wi in range(NW):
                b0 = int(slot_off[h, wi]) // 128
                wofb[b0 : b0 + int(B[h, wi])] = wi
            dst_node = wofb[None, :] * W_N + dof.astype(np.int64)
            valid = dof >= 0
            iv = np.where(valid, inv_pad[m][np.clip(dst_node, 0, NPAD - 1)], 0.0)
            pair.append(iv.astype(np.float32))
            if S_DRAM:
                oh = (
                    dof[:, :, None] == np.arange(W_N, dtype=np.float32)[None, None, :]
                )
                spair.append(
                    (oh * iv[:, :, None]).astype(np.float16)
                )
        invslot_arr.append(pair)
        sblk_arr.append(spair)

    invrow_arr = [
        np.broadcast_to(inv_pad[m].astype(np.float16)[None, :], (128, NPAD)).copy()
        for m in range(NC)
    ]

    return {
        "sblk": sblk_arr,
        "invrow": invrow_arr,
        "B": B,
        "slots_h": slots_h,
        "nb_h": nb_h,
        "slot_off": slot_off,
        "chunks": chunks,
        "idx_wrapped": idx_wrapped,
        "dof": dof_arr,
        "invslot": invslot_arr,
    }


def _build_program(struct, timing_reps=1):
    B = struct["B"]
    slots_h = struct["slots_h"]
    nb_h = struct["nb_h"]
    slot_off = struct["slot_off"]
    chunks = struct["chunks"]

    nc = bacc.Bacc(
        "TRN2",
        target_bir_lowering=False,
        debug=False,
        num_devices=NC,
        num_swdge_queues=NQ,
        dynamic_dma_scratch_size=DMA_SCRATCH,
    )
    f32, f16, i16 = mybir.dt.float32, mybir.dt.float16, mybir.dt.int16
    f8 = mybir.dt.float8e4

    idx_d = [
        nc.dram_tensor(f"idx{h}", [128, max(slots_h[h] // 16, 1)], i16, kind="ExternalInput")
        for h in range(2)
    ]
    dof_d = [
        nc.dram_tensor(f"dof{h}", [128, max(nb_h[h], 1)], f32, kind="ExternalInput")
        for h in range(2)
    ]
    xT_d = nc.dram_tensor("xT", [KCH, 128, NPAD], f16, kind="ExternalInput")
    embW_d = nc.dram_tensor("embW", [KCH, 128, HID], f16, kind="ExternalInput")
    embB_d = nc.dram_tensor("embB", [128, 1], f32, kind="ExternalInput")
    Wl_d = nc.dram_tensor("Wl", [L, 128, HID], f16, kind="ExternalInput")
    Wr_d = nc.dram_tensor("Wr", [L, 128, HID], f16, kind="ExternalInput")
    bl_d = nc.dram_tensor("bl", [L, 128, 1], f32, kind="ExternalInput")
    iota_d = nc.dram_tensor("iota", [128, W_N], f16, kind="ExternalInput")
    invrow_d = nc.dram_tensor("invrow", [128, NPAD], f16, kind="ExternalInput")
    inval_d = [
        nc.dram_tensor(f"inval{h}", [128, max(nb_h[h], 1)], f32, kind="ExternalInput")
        for h in range(2)
    ]
    sblk_d = [
        nc.dram_tensor(f"sblk{h}", [128, max(nb_h[h], 1), W_N], f16, kind="ExternalInput")
        for h in range(2)
    ] if S_DRAM else None
    out_d = nc.dram_tensor("out", [NPAD, HID], f32, kind="ExternalOutput")

    rg = [list(range(NC))]
    qctr = [0]

    def next_q():
        q = qctr[0] % NQ
        qctr[0] += 1
        return q

    import os as _os
    _trace = _os.environ.get("KERNEL_TRACE_SIM") == "1"
    _ablate = _os.environ.get("KERNEL_ABLATE") == "1"
    _no_ag = _os.environ.get("KERNEL_NO_AG") == "1"
    _no_gather = _os.environ.get("KERNEL_NO_GATHER") == "1"
    _no_s = _os.environ.get("KERNEL_NO_S") == "1"
    with tile.TileContext(nc, trace_sim=_trace) as tc:
        with (
            tc.tile_pool(name="const", bufs=1) as constp,
            tc.tile_pool(name="big", bufs=1) as bigp,
            tc.tile_pool(name="gt", bufs=GT_BUFS) as gtp,
            tc.tile_pool(name="sp", bufs=GT_BUFS) as sp,
            tc.tile_pool(name="ap", bufs=4) as apool,
            tc.tile_pool(name="hp", bufs=4) as hpool,
            tc.tile_pool(name="pag", bufs=PAG_BUFS, space="PSUM") as pag,
            tc.tile_pool(name="pz", bufs=2, space="PSUM") as pz,
            tc.tile_pool(name="pt", bufs=PT_BUFS, space="PSUM") as pt,
            tc.tile_pool(name="dram", bufs=1, space="DRAM") as dram,
        ):
            nc.gpsimd.load_library(library_config.mlp)

            # --- resident constants / inputs in SBUF ---
            idx_sb = []
            dof_sb = []
            for h in range(2):
                t = constp.tile([128, max(slots_h[h] // 16, 1)], i16, name=f"idxsb{h}")
                nc.sync.dma_start(out=t[:], in_=idx_d[h][:])
                idx_sb.append(t)
                t2 = constp.tile([128, max(nb_h[h], 1)], f32, name=f"dofsb{h}")
                nc.sync.dma_start(out=t2[:], in_=dof_d[h][:])
                dof_sb.append(t2)
            inval_sb = []
            if S_MODE in ("tt", "ts"):
                for h in range(2):
                    t3 = constp.tile(
                        [128, max(nb_h[h], 1)], f32, name=f"invalsb{h}"
                    )
                    nc.sync.dma_start(out=t3[:], in_=inval_d[h][:])
                    inval_sb.append(t3)
            iota_sb = constp.tile([128, W_N], f16)
            nc.sync.dma_start(out=iota_sb[:], in_=iota_d[:])
            invrow_sb = constp.tile([128, NPAD], f16)
            nc.sync.dma_start(out=invrow_sb[:], in_=invrow_d[:])
            ident = constp.tile([128, 128], f32)
            make_identity(nc, ident[:])
            ident16 = constp.tile([128, 128], f16)
            nc.vector.tensor_copy(ident16[:], ident[:])
            embW_sb = constp.tile([128, KCH, HID], f16)
            nc.sync.dma_start(out=embW_sb[:], in_=embW_d[:].rearrange("k p h -> p k h"))
            embB_sb = constp.tile([128, 1], f32)
            nc.sync.dma_start(out=embB_sb[:], in_=embB_d[:])
            Wl_sb = constp.tile([128, L, HID], f16)
            nc.sync.dma_start(out=Wl_sb[:], in_=Wl_d[:].rearrange("l p h -> p l h"))
            Wr_sb = constp.tile([128, L, HID], f16)
            nc.sync.dma_start(out=Wr_sb[:], in_=Wr_d[:].rearrange("l p h -> p l h"))
            bl_sb = constp.tile([128, L], f32)
            nc.sync.dma_start(out=bl_sb[:], in_=bl_d[:].rearrange("l p one -> p (l one)"))
            hT = [bigp.tile([128, NPAD], f16, name=f"hT{i}") for i in range(2)]
            partA = bigp.tile([128, NPAD], f16, name="partA")  # phase-A means

            # DRAM buffers (fp16 table + per-layer AllGather outputs)
            n_ag = 1 + timing_reps * (L - 1)
            cc_in = [
                dram.tile([NPAD, HID], f16, name=f"ccin{i}", bufs=1) for i in range(2)
            ]
            h_all = [
                [
                    dram.tile(
                        [HALF, HID], f16, name=f"hall{i}_{k}", bufs=1,
                        addr_space="Shared",
                    )
                    for k in range(2)
                ]
                for i in range(n_ag)
            ]

            def writeback(hTbuf, w, dest):
                # transpose window back to node-major (fp16) and DMA to dest rows
                cs = slice(w * W_N, (w + 1) * W_N)
                ptile = pt.tile([128, 128], f16, tag="pt16", name="ptile")
                nc.tensor.transpose(ptile[:], hTbuf[:, cs], ident16[:])
                hsb = hpool.tile([128, 128], f16, tag="hsb", name="hsb")
                nc.scalar.activation(
                    hsb[:], ptile[:], mybir.ActivationFunctionType.Identity
                )
                nc.sync.dma_start(out=dest[cs, :], in_=hsb[:])

            def allgather(src_cc, dst_pair, k):
                ins_ap = src_cc[k * RCH : (k + 1) * RCH, :]
                if _no_ag:
                    nc.sync.dma_start(out=dst_pair[k][0:RCH, :], in_=ins_ap)
                    return
                nc.gpsimd.collective_compute(
                    "AllGather",
                    mybir.AluOpType.bypass,
                    replica_groups=rg,
                    ins=[ins_ap],
                    outs=[dst_pair[k][:]],
                )

            def embedding(xT_sb):
                for w in range(NW):
                    ws = slice(w * W_N, (w + 1) * W_N)
                    pzz = pz.tile([128, W_N], f32, tag="pz", name="pz")
                    for k in range(KCH):
                        nc.tensor.matmul(
                            pzz[:],
                            lhsT=embW_sb[:, k, :],
                            rhs=xT_sb[:, k, ws],
                            start=(k == 0),
                            stop=(k == KCH - 1),
                        )
                    nc.scalar.activation(
                        hT[0][:, ws], pzz[:], mybir.ActivationFunctionType.Relu,
                        bias=embB_sb[:],
                    )
                    writeback(hT[0], w, cc_in[0])
                    if w == NW // 2 - 1:
                        allgather(cc_in[0], h_all[0], 0)
                allgather(cc_in[0], h_all[0], 1)

            def layer(l, h_src, hT_in, hT_out, dest, next_hall, S_all=None):
                half_ap = [h_src[0][:], h_src[1][:]]
                issued = [{}, {}]
                nexti = [0, 0]

                def ensure_chunk(h, ci):
                    while nexti[h] <= ci:
                        w0, n = chunks[h][nexti[h]]
                        nb = n // 128
                        sdt = f8 if S_MODE == "f8" else f16
                        gt = gtp.tile([128, nb, 128], f16, tag="gt", name="gt")
                        if _no_gather:
                            nc.vector.memset(gt[:], 0.0)
                        else:
                            nc.gpsimd.dma_gather(
                                gt[:],
                                half_ap[h],
                                idx_sb[h][:, w0 // 16 : (w0 + n) // 16],
                                n,
                                n,
                                HID,
                                queue_num=next_q(),
                            )
                        if S_MODE == "pf8":
                            # persistent layer-invariant S; no per-chunk build
                            issued[h][nexti[h]] = (gt, None)
                            nexti[h] += 1
                            continue
                        # batched one-hot S for all nb blocks of this chunk,
                        # scaled per-slot by 1/deg(dst) (mean aggregation)
                        b0 = w0 // 128
                        S = sp.tile([128, nb, W_N], sdt, tag="S", name="S")
                        if _no_s:
                            nc.vector.memset(S[:], 0.0)
                        elif S_MODE == "f8":
                            # pure one-hot in fp8 (exact); 1/deg applied to the
                            # flushed partials instead -> single DVE pass.
                            nc.vector.tensor_tensor(
                                out=S[:],
                                in0=dof_sb[h][:, b0 : b0 + nb, None].broadcast_to(
                                    [128, nb, W_N]
                                ),
                                in1=iota_sb[:, None, :].broadcast_to([128, nb, W_N]),
                                op=mybir.AluOpType.is_equal,
                            )
                        elif S_MODE == "dram":
                            # S blocks precomputed on host (pure edge_index
                            # preprocessing); stream from DRAM, no DVE work.
                            nc.sync.dma_start(
                                out=S[:], in_=sblk_d[h][:, b0 : b0 + nb, :]
                            )
                        elif S_MODE == "ts":
                            # fused one-hot build: S[:,j,:] = (iota == dof_j) * inval_j
                            # per-partition scalars keep all tensor operands
                            # packed f16/SBUF -> DVE 4x_2p fast mode.
                            for j in range(nb):
                                nc.vector.tensor_scalar(
                                    out=S[:, j, :],
                                    in0=iota_sb[:],
                                    scalar1=dof_sb[h][:, b0 + j : b0 + j + 1],
                                    scalar2=inval_sb[h][:, b0 + j : b0 + j + 1],
                                    op0=mybir.AluOpType.is_equal,
                                    op1=mybir.AluOpType.mult,
                                )
                        else:
                            nc.vector.tensor_tensor(
                                out=S[:],
                                in0=dof_sb[h][:, b0 : b0 + nb, None].broadcast_to(
                                    [128, nb, W_N]
                                ),
                                in1=iota_sb[:, None, :].broadcast_to([128, nb, W_N]),
                                op=mybir.AluOpType.is_equal,
                            )
                            nc.vector.tensor_tensor(
                                out=S[:],
                                in0=S[:],
                                in1=inval_sb[h][:, b0 : b0 + nb, None].broadcast_to(
                                    [128, nb, W_N]
                                ),
                                op=mybir.AluOpType.mult,
                            )
                        issued[h][nexti[h]] = (gt, S)
                        nexti[h] += 1

                def agg_blocks(h, w, pagg):
                    nb = int(B[h][w])
                    for b in range(nb):
                        ab = int(slot_off[h][w]) + b * 128
                        ci = ab // GMAX
                        ensure_chunk(h, ci)
                        if _ablate:
                            continue
                        gt, S = issued[h][ci]
                        j = (ab % GMAX) // 128
                        rhs = (
                            S_all[h][:, ab // 128, :]
                            if S_MODE == "pf8"
                            else S[:, j, :]
                        )
                        nc.tensor.matmul(
                            pagg[:],
                            lhsT=gt[:, j, :],
                            rhs=rhs,
                            start=(b == 0),
                            stop=(b == nb - 1),
                        )
                    return nb > 0 and not _ablate

                # ---- phase A: half-0 partial means for all windows ----
                if S_MODE in ("f8", "pf8"):
                    # windows processed in pairs sharing one PSUM bank with a
                    # single fused flush+scale per pair: halves the number of
                    # PSUM->SBUF round-trips (sem chains) in phase A.
                    GRP = 4 if NW % 4 == 0 else 2
                    for wp in range(0, NW, GRP):
                        pagg2 = pag.tile(
                            [128, GRP, W_N], f32, tag="pagg", name="pagg"
                        )
                        got = [
                            agg_blocks(0, wp + k, pagg2[:, k, :])
                            for k in range(GRP)
                        ]
                        ws2 = slice(wp * W_N, (wp + GRP) * W_N)
                        if all(got):
                            nc.vector.tensor_tensor(
                                out=partA[:, ws2],
                                in0=pagg2[:].rearrange("p a b -> p (a b)"),
                                in1=invrow_sb[:, ws2],
                                op=mybir.AluOpType.mult,
                            )
                        else:
                            for k in range(GRP):
                                wk = slice(
                                    (wp + k) * W_N, (wp + k + 1) * W_N
                                )
                                if got[k]:
                                    nc.vector.tensor_tensor(
                                        out=partA[:, wk],
                                        in0=pagg2[:, k, :],
                                        in1=invrow_sb[:, wk],
                                        op=mybir.AluOpType.mult,
                                    )
                                else:
                                    nc.vector.memset(partA[:, wk], 0.0)
                else:
                    for w in range(NW):
                        ws = slice(w * W_N, (w + 1) * W_N)
                        pagg = pag.tile([128, W_N], f32, tag="paggs", name="pagg")
                        if agg_blocks(0, w, pagg):
                            nc.scalar.activation(
                                partA[:, ws], pagg[:],
                                mybir.ActivationFunctionType.Identity,
                            )
                        else:
                            nc.vector.memset(partA[:, ws], 0.0)

                # ---- phase B: half-1 + dense + writeback ----
                # the writeback chain (PE transpose -> ACT/DVE copy -> DMA) of
                # window w is issued one window late so the in-order PE queue
                # never stalls on window w's relu before starting w+1's aggs.
                pending = []

                def flush_pending():
                    while pending:
                        pending.pop(0)()

                if S_MODE == "pf8":
                    # strip-dense: phase-B partials are ADDED into partA on the
                    # (idle) DVE, then the dense transform runs in 512-wide
                    # strips: 2 matmuls + 1 relu per strip instead of 3+1 per
                    # 128-window -> ~250 fewer PE insts and 37 fewer ACT insts
                    # per layer.
                    SW = 512
                    for s0 in range(0, NPAD, SW):
                        sw = min(SW, NPAD - s0)
                        for w in range(s0 // W_N, (s0 + sw) // W_N):
                            ws = slice(w * W_N, (w + 1) * W_N)
                            pagg = pag.tile([128, W_N], f32, tag="paggs", name="pagg")
                            if agg_blocks(1, w, pagg):
                                tmp = apool.tile([128, W_N], f16, tag="p2", name="p2")
                                nc.vector.tensor_tensor(
                                    out=tmp[:], in0=pagg[:],
                                    in1=invrow_sb[:, ws], op=mybir.AluOpType.mult,
                                )
                                nc.vector.tensor_tensor(
                                    out=partA[:, ws], in0=partA[:, ws],
                                    in1=tmp[:], op=mybir.AluOpType.add,
                                )
                        while len(pending) > 1:
                            pending.pop(0)()
                        ss = slice(s0, s0 + sw)
                        pzz = pz.tile([128, SW], f32, tag="pz", name="pz")
                        nc.tensor.matmul(
                            pzz[:, :sw], lhsT=Wl_sb[:, l, :], rhs=partA[:, ss],
                            start=True, stop=False,
                        )
                        nc.tensor.matmul(
                            pzz[:, :sw], lhsT=Wr_sb[:, l, :], rhs=hT_in[:, ss],
                            start=False, stop=True,
                        )
                        if l < L - 1:
                            nc.scalar.activation(
                                hT_out[:, ss], pzz[:, :sw],
                                mybir.ActivationFunctionType.Relu,
                                bias=bl_sb[:, l : l + 1],
                            )

                            def wb(s0=s0, sw=sw):
                                for w in range(s0 // W_N, (s0 + sw) // W_N):
                                    writeback(hT_out, w, dest)
                                    if w == NW // 2 - 1:
                                        allgather(dest, next_hall, 0)
                                    elif w == NW - 1:
                                        allgather(dest, next_hall, 1)

                            pending.append(wb)
                        else:
                            h4s = apool.tile([128, SW], f16, tag="h4s", name="h4s")
                            nc.scalar.activation(
                                h4s[:, :sw], pzz[:, :sw],
                                mybir.ActivationFunctionType.Relu,
                                bias=bl_sb[:, l : l + 1],
                            )

                            def wb(s0=s0, sw=sw, h4s=h4s):
                                for wi, w in enumerate(
                                    range(s0 // W_N, (s0 + sw) // W_N)
                                ):
                                    cs = slice(w * W_N, (w + 1) * W_N)
                                    ptile = pt.tile(
                                        [128, 128], f16, tag="pt16", name="ptile"
                                    )
                                    nc.tensor.transpose(
                                        ptile[:],
                                        h4s[:, wi * 128 : (wi + 1) * 128],
                                        ident16[:],
                                    )
                                    hsb = hpool.tile(
                                        [128, 128], f32, tag="hsbo", name="hsbo"
                                    )
                                    nc.vector.tensor_copy(hsb[:], ptile[:])
                                    nc.sync.dma_start(out=out_d[cs, :], in_=hsb[:])

                            pending.append(wb)
                    flush_pending()
                    return

                def dense_relu_wb(w, got, part2_ap):
                    ws = slice(w * W_N, (w + 1) * W_N)
                    pzz = pz.tile([128, W_N], f32, tag="pz", name="pz")
                    nc.tensor.matmul(
                        pzz[:], lhsT=Wl_sb[:, l, :], rhs=partA[:, ws],
                        start=True, stop=False,
                    )
                    if got:
                        nc.tensor.matmul(
                            pzz[:], lhsT=Wl_sb[:, l, :], rhs=part2_ap,
                            start=False, stop=False,
                        )
                    nc.tensor.matmul(
                        pzz[:], lhsT=Wr_sb[:, l, :], rhs=hT_in[:, ws], start=False,
                        stop=True,
                    )
                    if l < L - 1:
                        nc.scalar.activation(
                            hT_out[:, ws], pzz[:],
                            mybir.ActivationFunctionType.Relu,
                            bias=bl_sb[:, l : l + 1],
                        )

                        def wb(w=w):
                            writeback(hT_out, w, dest)
                            if w == NW // 2 - 1:
                                allgather(dest, next_hall, 0)
                            elif w == NW - 1:
                                allgather(dest, next_hall, 1)

                        pending.append(wb)
                    else:
                        h4 = apool.tile([128, W_N], f16, tag="h4", name="h4")
                        nc.scalar.activation(
                            h4[:], pzz[:],
                            mybir.ActivationFunctionType.Relu,
                            bias=bl_sb[:, l : l + 1],
                        )

                        def wb(w=w, h4=h4):
                            cs = slice(w * W_N, (w + 1) * W_N)
                            ptile = pt.tile([128, 128], f16, tag="pt16", name="ptile")
                            nc.tensor.transpose(ptile[:], h4[:], ident16[:])
                            hsb = hpool.tile([128, 128], f32, tag="hsbo", name="hsbo")
                            nc.vector.tensor_copy(hsb[:], ptile[:])
                            nc.sync.dma_start(out=out_d[cs, :], in_=hsb[:])

                        pending.append(wb)
                    if len(pending) > 1:
                        pending.pop(0)()

                if S_MODE in ("f8", "pf8"):
                    # paired phase-B: two windows share one PSUM bank and one
                    # fused flush+scale, then each window's dense runs.
                    GRP = 4 if NW % 4 == 0 else 2
                    for wp in range(0, NW, GRP):
                        pagg2b = pag.tile(
                            [128, GRP, W_N], f32, tag="pagg", name="pagg"
                        )
                        gots = [
                            agg_blocks(1, wp + k, pagg2b[:, k, :])
                            for k in range(GRP)
                        ]
                        part2 = apool.tile(
                            [128, GRP, W_N], f16, tag="p2", name="p2"
                        )
                        ws2 = slice(wp * W_N, (wp + GRP) * W_N)
                        if all(gots):
                            nc.vector.tensor_tensor(
                                out=part2[:].rearrange("p a b -> p (a b)"),
                                in0=pagg2b[:].rearrange("p a b -> p (a b)"),
                                in1=invrow_sb[:, ws2],
                                op=mybir.AluOpType.mult,
                            )
                        else:
                            for k in range(GRP):
                                if gots[k]:
                                    wk = slice(
                                        (wp + k) * W_N, (wp + k + 1) * W_N
                                    )
                                    nc.vector.tensor_tensor(
                                        out=part2[:, k, :],
                                        in0=pagg2b[:, k, :],
                                        in1=invrow_sb[:, wk],
                                        op=mybir.AluOpType.mult,
                                    )
                        for k in range(GRP):
                            dense_relu_wb(wp + k, gots[k], part2[:, k, :])
                    flush_pending()
                    return

                for w in range(NW):
                    ws = slice(w * W_N, (w + 1) * W_N)
                    pagg = pag.tile([128, W_N], f32, tag="paggs", name="pagg")
                    got = agg_blocks(1, w, pagg)
                    if got:
                        part2 = apool.tile([128, W_N], f16, tag="p2s", name="p2")
                        nc.scalar.activation(
                            part2[:], pagg[:],
                            mybir.ActivationFunctionType.Identity,
                        )
                    pzz = pz.tile([128, W_N], f32, tag="pz", name="pz")
                    nc.tensor.matmul(
                        pzz[:], lhsT=Wl_sb[:, l, :], rhs=partA[:, ws],
                        start=True, stop=False,
                    )
                    if got:
                        nc.tensor.matmul(
                            pzz[:], lhsT=Wl_sb[:, l, :], rhs=part2[:],
                            start=False, stop=False,
                        )
                    nc.tensor.matmul(
                        pzz[:], lhsT=Wr_sb[:, l, :], rhs=hT_in[:, ws], start=False,
                        stop=True,
                    )
                    if l < L - 1:
                        nc.scalar.activation(
                            hT_out[:, ws], pzz[:],
                            mybir.ActivationFunctionType.Relu,
                            bias=bl_sb[:, l : l + 1],
                        )

                        def wb(w=w):
                            writeback(hT_out, w, dest)
                            if w == NW // 2 - 1:
                                allgather(dest, next_hall, 0)
                            elif w == NW - 1:
                                allgather(dest, next_hall, 1)

                        pending.append(wb)
                    else:
                        h4 = apool.tile([128, W_N], f16, tag="h4", name="h4")
                        nc.scalar.activation(
                            h4[:], pzz[:],
                            mybir.ActivationFunctionType.Relu,
                            bias=bl_sb[:, l : l + 1],
                        )

                        def wb(w=w, h4=h4):
                            cs = slice(w * W_N, (w + 1) * W_N)
                            ptile = pt.tile([128, 128], f16, tag="pt16", name="ptile")
                            nc.tensor.transpose(ptile[:], h4[:], ident16[:])
                            hsb = hpool.tile([128, 128], f32, tag="hsbo", name="hsbo")
                            nc.vector.tensor_copy(hsb[:], ptile[:])
                            nc.sync.dma_start(out=out_d[cs, :], in_=hsb[:])

                        pending.append(wb)
                    if len(pending) > 1:
                        pending.pop(0)()
                flush_pending()

            import os as _os2
            _noscope = _os2.environ.get("KERNEL_PF8_NOSCOPE", "0") == "1"
            if _noscope:
                xT_sb = bigp.tile([128, KCH, NPAD], f16, name="xT")
                nc.sync.dma_start(
                    out=xT_sb[:], in_=xT_d[:].rearrange("k p n -> p k n")
                )
                embedding(xT_sb)
            else:
                with tc.tile_pool(name="xt", bufs=1) as xtp:
                    xT_sb = xtp.tile([128, KCH, NPAD], f16)
                    nc.sync.dma_start(
                        out=xT_sb[:], in_=xT_d[:].rearrange("k p n -> p k n")
                    )
                    embedding(xT_sb)

            def run_layers(S_all=None):
                agi = 0
                for rep in range(timing_reps):
                    for l in range(L):
                        layer(
                            l,
                            h_all[agi],
                            hT[l % 2],
                            hT[(l + 1) % 2],
                            cc_in[(l + 1) % 2],
                            h_all[agi + 1] if l < L - 1 else None,
                            S_all=S_all,
                        )
                        if l < L - 1:
                            agi += 1

            if S_MODE == "pf8":
                # layer-invariant one-hot S built once in fp8, resident in the
                # SBUF region vacated by the embedding's xT tile
                with tc.tile_pool(name="sall", bufs=1) as sallp:
                    S_all = [
                        sallp.tile(
                            [128, max(nb_h[h], 1), W_N], f8, name=f"sall{h}"
                        )
                        for h in range(2)
                    ]
                    for h in range(2):
                        for w0, n in chunks[h]:
                            nb = n // 128
                            b0 = w0 // 128
                            nc.vector.tensor_tensor(
                                out=S_all[h][:, b0 : b0 + nb, :],
                                in0=dof_sb[h][:, b0 : b0 + nb, None].broadcast_to(
                                    [128, nb, W_N]
                                ),
                                in1=iota_sb[:, None, :].broadcast_to(
                                    [128, nb, W_N]
                                ),
                                op=mybir.AluOpType.is_equal,
                            )
                    run_layers(S_all)
            else:
                run_layers()

    nc.compile()
    return nc


def _prep_inputs(inputs, struct):
    x = np.asarray(inputs["x"], dtype=np.float32)
    emb_W = np.asarray(inputs["emb_W"], dtype=np.float32)
    emb_b = np.asarray(inputs["emb_b"], dtype=np.float32)
    Wl = np.asarray(inputs["Wl"], dtype=np.float32)
    bl = np.asarray(inputs["bl"], dtype=np.float32)
    Wr = np.asarray(inputs["Wr"], dtype=np.float32)

    embW_p = np.zeros((KCH, 128, HID), dtype=np.float16)
    embW_p.reshape(KCH * 128, HID)[:IN_DIM] = emb_W.astype(np.float16)
    embB_p = np.zeros((128, 1), dtype=np.float32)
    embB_p[:, 0] = emb_b
    Wl_p = Wl.astype(np.float16)
    Wr_p = Wr.astype(np.float16)
    bl_p = np.ascontiguousarray(bl[:, :, None].astype(np.float32))

    iota = np.broadcast_to(
        np.arange(W_N, dtype=np.float16)[None, :], (128, W_N)
    ).copy()

    in_maps = []
    for m in range(NC):
        xm = np.zeros((KCH * 128, NPAD), dtype=np.float16)
        xm[:IN_DIM, :NPC] = x[m * NPC : (m + 1) * NPC].T.astype(np.float16)
        im = {
            "idx0": struct["idx_wrapped"][m][0],
            "idx1": struct["idx_wrapped"][m][1],
            "dof0": struct["dof"][m][0],
            "dof1": struct["dof"][m][1],
            "inval0": struct["invslot"][m][0],
            "inval1": struct["invslot"][m][1],
            **(
                {"sblk0": struct["sblk"][m][0], "sblk1": struct["sblk"][m][1]}
                if S_DRAM
                else {}
            ),
            "xT": xm.reshape(KCH, 128, NPAD),
            "embW": embW_p,
            "embB": embB_p,
            "Wl": Wl_p,
            "Wr": Wr_p,
            "bl": bl_p,
            "iota": iota,
            "invrow": struct["invrow"][m],
        }
        in_maps.append(im)
    return in_maps


class BassRunner:
    """Executes a compiled Bass program via PJRT/axon; jit built once."""

    def __init__(self, nc, n_cores):
        import jax
        from jax.sharding import Mesh, PartitionSpec
        from jax.experimental.shard_map import shard_map
        from concourse.bass2jax import (
            _bass_exec_p,
            install_neuronx_cc_hook,
            partition_id_tensor,
        )

        install_neuronx_cc_hook()
        self.jax = jax
        self.nc = nc
        self.n_cores = n_cores
        partition_name = (
            nc.partition_id_tensor.name if nc.partition_id_tensor else None
        )
        in_names, out_names, out_avals, zero_outs = [], [], [], []
        for alloc in nc.m.functions[0].allocations:
            if not isinstance(alloc, mybir.MemoryLocationSet):
                continue
            name = alloc.memorylocations[0].name
            if alloc.kind == "ExternalInput":
                if name != partition_name:
                    in_names.append(name)
            elif alloc.kind == "ExternalOutput":
                shape = tuple(alloc.tensor_shape)
                dtype = mybir.dt.np(alloc.dtype)
                out_names.append(name)
                out_avals.append(jax.core.ShapedArray(shape, dtype))
                zero_outs.append(np.zeros(shape, dtype))
        self.in_names, self.out_names = in_names, out_names
        self.zero_outs, self._out_avals = zero_outs, out_avals
        n_params, n_outs = len(in_names), len(out_avals)
        all_in_names = in_names + out_names
        if partition_name is not None:
            all_in_names = all_in_names + [partition_name]

        def _body(*args):
            operands = list(args)
            if partition_name is not None:
                operands.append(partition_id_tensor())
            return tuple(
                _bass_exec_p.bind(
                    *operands,
                    out_avals=tuple(out_avals),
                    in_names=tuple(all_in_names),
                    out_names=tuple(out_names),
                    lowering_input_output_aliases=(),
                    sim_require_finite=True,
                    sim_require_nnan=True,
                    nc=nc,
                )
            )

        devices = jax.devices()[:n_cores]
        self._mesh = Mesh(np.asarray(devices), ("core",))
        self._pspec = PartitionSpec("core")
        in_specs = (self._pspec,) * (n_params + n_outs)
        out_specs = (self._pspec,) * len(out_names)
        self._fn = jax.jit(
            shard_map(
                _body,
                mesh=self._mesh,
                in_specs=in_specs,
                out_specs=out_specs,
                check_rep=False,
            ),
            keep_unused=True,
        )

    def prepare(self, in_maps):
        n = self.n_cores
        concat_in = [
            np.concatenate(
                [np.asarray(in_maps[c][name]) for c in range(n)], axis=0
            )
            for name in self.in_names
        ]
        concat_zeros = [
            np.zeros((n * z.shape[0], *z.shape[1:]), z.dtype)
            for z in self.zero_outs
        ]
        sharding = self.jax.sharding.NamedSharding(self._mesh, self._pspec)
        self._args = [
            self.jax.device_put(a, sharding) for a in concat_in + concat_zeros
        ]

    def execute(self):
        outs = self._fn(*self._args)
        self.jax.block_until_ready(outs)
        return outs

    def run(self):
        outs = self.execute()
        n = self.n_cores
        return [
            {
                name: np.asarray(outs[i]).reshape(
                    n, *self._out_avals[i].shape
                )[c]
                for i, name in enumerate(self.out_names)
            }
            for c in range(n)
        ]


def _get_runner(edge_index, timing_reps=1):
    import os as _os
    _flags = (
        _os.environ.get("KERNEL_NO_AG", ""),
        _os.environ.get("KERNEL_ABLATE", ""),
        _os.environ.get("KERNEL_NO_GATHER", ""),
        _os.environ.get("KERNEL_NO_S", ""),
        GMAX,
        DMA_SCRATCH,
        S_MODE,
        PAG_BUFS,
        GT_BUFS,
        NQ,
        PT_BUFS,
    )
    key = ("prog", timing_reps, _flags, hash(edge_index.tobytes()))
    if key in _CACHE:
        return _CACHE[key]
    struct = _host_prep(edge_index)
    nc = _build_program(struct, timing_reps=timing_reps)
    runner = BassRunner(nc, NC)
    _CACHE[key] = (struct, runner)
    return struct, runner


def kernel(**inputs):
    edge_index = np.asarray(inputs["edge_index"])
    struct, runner = _get_runner(edge_index)
    in_maps = _prep_inputs(inputs, struct)
    runner.prepare(in_maps)
    results = runner.run()
    out = np.empty((N, HID), dtype=np.float32)
    for m in range(NC):
        out[m * NPC : (m + 1) * NPC] = results[m]["out"][:NPC]
    return out



# revision 56
# speedup vs baseline: 1.0490x; 1.0027x over previous
"""GraphSAGE (mean aggregation) on 8 Trainium2 NeuronCores.

v7 additions on top of v6 (default S_MODE="f8"): S matrices are pure
one-hots in fp8e4 (exact 1.0/0.0, so no precision loss feeding the f16 PE
matmuls) built in a single DVE is_equal pass; the mean 1/deg scaling moved
out of S into a fused DVE flush (PSUM f32 * invrow -> f16 SBUF) replacing
the ACT Identity flush; phase-B writeback chains are issued one window late;
aggregation windows are processed in PAIRS sharing one PSUM bank with a
single fused flush+scale per pair (halves PSUM->SBUF sem round-trips in
both phases). Measured HW total ~1.59-1.77 ms vs 1.82 ms baseline (axon
1x-vs-5x differencing, +-70 us noise; best observed 1.56 ms). Alternate modes kept for experiments:
"tt" (v6 two-pass f16 S), "ts" (per-block fused tensor_scalar — 4x slower
on HW), "dram" (host-built S streamed — slower), "pf8" (+KERNEL_PF8_NOSCOPE=1
KERNEL_GT=4: layer-invariant persistent S + 512-wide strip dense — neutral).

Strategy (v6):
  - Nodes partitioned across 8 cores (6250 real + pad -> 6400/core).
  - Full node-feature table h_all [51200, 128] fp16 replicated in each core's
    DRAM, laid out CHUNK-MAJOR: table row = h*25600 + m*3200 + (r - h*3200)
    for core m, local row r, chunk/half h = (r >= 3200). The per-layer
    AllGather is split into 2 contiguous chunk AllGathers, each issued as soon
    as the windows feeding it are written back -> chunk 0 of the next table
    overlaps the tail of the current layer, and the next layer's half-0
    gathers overlap chunk 1's AllGather.
  - Layers run in two phases: phase A aggregates half-0 edges for all windows
    (partial means flushed PSUM->SBUF f16 via the ACT engine), phase B adds
    half-1 edges; the dense SAGE transform accumulates Wl@partA + Wl@part2 +
    Wr@hT in PSUM, then relu (ACT), PE transpose to node-major, cc writeback.
  - Edge messages fetched with dma_gather (custom SWDGE gather, 4 queues,
    int16 indices, 1024-slot chunks spanning windows, slots sorted by src
    within each window for DRAM locality).
  - Mean aggregation = PE matmuls with one-hot S matrices batch-built on DVE
    per gather chunk; S carries the host-precomputed 1/deg(dst) scaling, so
    no per-window DVE combine work remains.
"""
import sys

sys.path.insert(0, "/opt/trn_rl_repo")

import numpy as np

import concourse.bass as bass
import concourse.bacc as bacc
import concourse.tile as tile
from concourse import mybir, library_config
from concourse.masks import make_identity

# problem constants (hardcoded per contract)
N, E, IN_DIM, HID, L = 50000, 625000, 300, 128, 4
NC = 8
NPC = N // NC            # 6250 real nodes per core
W_N = 128                # aggregation window width (psum free dim)
NW = 50                  # windows per core
NPAD = W_N * NW          # 6400 padded nodes per core
NTAB = NC * NPAD         # 51200 rows in the replicated table
HALF = NTAB // 2         # 25600 rows per table chunk (int16 idx limit)
RCH = NPAD // 2          # 3200 local rows per AllGather chunk
KCH = 3                  # 384 = 3*128 >= IN_DIM contraction chunks
import os as _os_mod
GMAX = int(_os_mod.environ.get("KERNEL_GMAX", "1024"))  # slots per dma_gather
DMA_SCRATCH = int(_os_mod.environ.get("KERNEL_DMA_SCRATCH", "16384"))
# S-build mode: "tt" = chunk-level tensor_tensor 2-pass (v6),
# "ts" = per-block fused tensor_scalar, "dram" = host-built, streamed
S_MODE = _os_mod.environ.get("KERNEL_S_MODE", "f8")
if _os_mod.environ.get("KERNEL_S_DRAM", "0") == "1":
    S_MODE = "dram"
S_DRAM = S_MODE == "dram"
PAG_BUFS = int(_os_mod.environ.get("KERNEL_PAG", "4"))
GT_BUFS = int(_os_mod.environ.get("KERNEL_GT", "6"))
NQ = int(_os_mod.environ.get("KERNEL_NQ", "4"))
PT_BUFS = int(_os_mod.environ.get("KERNEL_PT", "2"))

_CACHE = {}


def _host_prep(edge_index):
    """Build per-core gather streams, dst-offset blocks and program structure."""
    src = edge_index[0].astype(np.int64)
    dst = edge_index[1].astype(np.int64)
    # padded global ids
    gsrc = (src // NPC) * NPAD + (src % NPC)
    gdst = (dst // NPC) * NPAD + (dst % NPC)
    # chunk-major table index of each source
    m_s = gsrc // NPAD
    r_s = gsrc % NPAD
    h_s = (r_s >= RCH).astype(np.int64)
    tok_s = m_s * RCH + (r_s - h_s * RCH)      # 0..25599 within half

    per_core = []
    counts = np.zeros((NC, 2, NW), dtype=np.int64)
    for m in range(NC):
        sel = (gdst // NPAD) == m
        t_m = tok_s[sel]
        h_m = h_s[sel]
        dl = (gdst[sel] - m * NPAD).astype(np.int64)   # 0..6249
        w = dl // W_N
        # sort by (half, window, src-token) -> ascending DMA addresses
        order = np.lexsort((t_m, w, h_m))
        t_m, dl, h_m, w = t_m[order], dl[order], h_m[order], w[order]
        per_core.append((t_m, dl, h_m, w))
        for h in range(2):
            cw = np.bincount(w[h_m == h], minlength=NW)
            counts[m, h, :] = cw

    # uniform block structure across cores
    B = np.zeros((2, NW), dtype=np.int64)
    for h in range(2):
        for w in range(NW):
            B[h, w] = int(np.ceil(counts[:, h, w].max() / 128.0))

    slots_h = [int(B[h].sum() * 128) for h in range(2)]
    nb_h = [int(B[h].sum()) for h in range(2)]

    slot_off = np.zeros((2, NW), dtype=np.int64)
    for h in range(2):
        acc = 0
        for w in range(NW):
            slot_off[h, w] = acc
            acc += B[h, w] * 128

    # gather instruction chunks per half: (start_slot, n) spanning windows
    chunks = [[], []]
    for h in range(2):
        off = 0
        while off < slots_h[h]:
            n = min(GMAX, slots_h[h] - off)
            chunks[h].append((off, n))
            off += n

    idx_wrapped = []   # per core: [2][128, slots_h/16] int16
    dof_arr = []       # per core: [2][128, nb_h] fp32
    for m in range(NC):
        t_m, dl, h_m, w = per_core[m]
        iw_pair, dof_pair = [], []
        for h in range(2):
            tok = np.zeros(slots_h[h], dtype=np.int16)
            dof = np.full(slots_h[h], -1.0, dtype=np.float32)
            sel = h_m == h
            t_h, dl_h, w_h = t_m[sel], dl[sel], w[sel]
            for wi in range(NW):
                selw = w_h == wi
                cnt = int(selw.sum())
                if cnt == 0:
                    continue
                o = int(slot_off[h, wi])
                tok[o : o + cnt] = t_h[selw].astype(np.int16)
                dof[o : o + cnt] = (dl_h[selw] - wi * W_N).astype(np.float32)
            # wrap idx per gather instruction: j -> [j%16, j//16], replicated x8
            iw = np.zeros((128, slots_h[h] // 16), dtype=np.int16)
            for w0, n in chunks[h]:
                blockw = tok[w0 : w0 + n].reshape(n // 16, 16).T  # [16, n/16]
                iw[:, w0 // 16 : (w0 + n) // 16] = np.tile(blockw, (8, 1))
            iw_pair.append(iw)
            # dstoff partition-major: dof_arr[p, b] = dof[b*128+p]
            dof_pair.append(
                np.ascontiguousarray(dof.reshape(nb_h[h], 128).T).astype(np.float32)
            )
        idx_wrapped.append(iw_pair)
        dof_arr.append(dof_pair)

    # host-side inverse in-degree (pure edge_index preprocessing), folded
    # into per-slot scale values: invslot[p, b] = 1/deg(dst of slot b*128+p),
    # 0 for padding slots -> S matrices carry the mean scaling directly.
    deg = np.bincount(dst, minlength=N).astype(np.float32)
    inv = 1.0 / np.maximum(deg, 1.0)
    inv_pad = np.zeros((NC, NPAD), dtype=np.float32)
    inv_pad[:, :NPC] = inv.reshape(NC, NPC)

    invslot_arr = []  # per core: [2][128, nb_h] fp16
    sblk_arr = []     # per core: [2][128, nb_h, W_N] f16 host-built S blocks
    for m in range(NC):
        pair = []
        spair = []
        for h in range(2):
            dof = dof_arr[m][h].astype(np.float32)      # [128, nb]
            nb = dof.shape[1]
            # dst node of slot (p, b) = window(b)*W_N + dof
            wofb = np.zeros(nb, dtype=np.int64)
            for wi in range(NW):
                b0 = int(slot_off[h, wi]) // 128
                wofb[b0 : b0 + int(B[h, wi])] = wi
            dst_node = wofb[None, :] * W_N + dof.astype(np.int64)
            valid = dof >= 0
            iv = np.where(valid, inv_pad[m][np.clip(dst_node, 0, NPAD - 1)], 0.0)
            pair.append(iv.astype(np.float32))
            if S_DRAM:
                oh = (
                    dof[:, :, None] == np.arange(W_N, dtype=np.float32)[None, None, :]
                )
                spair.append(
                    (oh * iv[:, :, None]).astype(np.float16)
                )
        invslot_arr.append(pair)
        sblk_arr.append(spair)

    invrow_arr = [
        np.broadcast_to(inv_pad[m].astype(np.float16)[None, :], (128, NPAD)).copy()
        for m in range(NC)
    ]

    return {
        "sblk": sblk_arr,
        "invrow": invrow_arr,
        "B": B,
        "slots_h": slots_h,
        "nb_h": nb_h,
        "slot_off": slot_off,
        "chunks": chunks,
        "idx_wrapped": idx_wrapped,
        "dof": dof_arr,
        "invslot": invslot_arr,
    }


def _build_program(struct, timing_reps=1):
    B = struct["B"]
    slots_h = struct["slots_h"]
    nb_h = struct["nb_h"]
    slot_off = struct["slot_off"]
    chunks = struct["chunks"]

    nc = bacc.Bacc(
        "TRN2",
        target_bir_lowering=False,
        debug=False,
        num_devices=NC,
        num_swdge_queues=NQ,
        dynamic_dma_scratch_size=DMA_SCRATCH,
    )
    f32, f16, i16 = mybir.dt.float32, mybir.dt.float16, mybir.dt.int16
    f8 = mybir.dt.float8e4

    idx_d = [
        nc.dram_tensor(f"idx{h}", [128, max(slots_h[h] // 16, 1)], i16, kind="ExternalInput")
        for h in range(2)
    ]
    dof_d = [
        nc.dram_tensor(f"dof{h}", [128, max(nb_h[h], 1)], f32, kind="ExternalInput")
        for h in range(2)
    ]
    xT_d = nc.dram_tensor("xT", [KCH, 128, NPAD], f16, kind="ExternalInput")
    embW_d = nc.dram_tensor("embW", [KCH, 128, HID], f16, kind="ExternalInput")
    embB_d = nc.dram_tensor("embB", [128, 1], f32, kind="ExternalInput")
    Wl_d = nc.dram_tensor("Wl", [L, 128, HID], f16, kind="ExternalInput")
    Wr_d = nc.dram_tensor("Wr", [L, 128, HID], f16, kind="ExternalInput")
    bl_d = nc.dram_tensor("bl", [L, 128, 1], f32, kind="ExternalInput")
    iota_d = nc.dram_tensor("iota", [128, W_N], f16, kind="ExternalInput")
    invrow_d = nc.dram_tensor("invrow", [128, NPAD], f16, kind="ExternalInput")
    inval_d = [
        nc.dram_tensor(f"inval{h}", [128, max(nb_h[h], 1)], f32, kind="ExternalInput")
        for h in range(2)
    ]
    sblk_d = [
        nc.dram_tensor(f"sblk{h}", [128, max(nb_h[h], 1), W_N], f16, kind="ExternalInput")
        for h in range(2)
    ] if S_DRAM else None
    out_d = nc.dram_tensor("out", [NPAD, HID], f32, kind="ExternalOutput")

    rg = [list(range(NC))]
    qctr = [0]

    def next_q():
        q = qctr[0] % NQ
        qctr[0] += 1
        return q

    import os as _os
    _trace = _os.environ.get("KERNEL_TRACE_SIM") == "1"
    _ablate = _os.environ.get("KERNEL_ABLATE") == "1"
    _no_ag = _os.environ.get("KERNEL_NO_AG") == "1"
    _no_gather = _os.environ.get("KERNEL_NO_GATHER") == "1"
    _no_s = _os.environ.get("KERNEL_NO_S") == "1"
    with tile.TileContext(nc, trace_sim=_trace) as tc:
        with (
            tc.tile_pool(name="const", bufs=1) as constp,
            tc.tile_pool(name="big", bufs=1) as bigp,
            tc.tile_pool(name="gt", bufs=GT_BUFS) as gtp,
            tc.tile_pool(name="sp", bufs=GT_BUFS) as sp,
            tc.tile_pool(name="ap", bufs=4) as apool,
            tc.tile_pool(name="hp", bufs=4) as hpool,
            tc.tile_pool(name="pag", bufs=PAG_BUFS, space="PSUM") as pag,
            tc.tile_pool(name="pz", bufs=2, space="PSUM") as pz,
            tc.tile_pool(name="pt", bufs=PT_BUFS, space="PSUM") as pt,
            tc.tile_pool(name="dram", bufs=1, space="DRAM") as dram,
        ):
            nc.gpsimd.load_library(library_config.mlp)

            # --- resident constants / inputs in SBUF ---
            idx_sb = []
            dof_sb = []
            for h in range(2):
                t = constp.tile([128, max(slots_h[h] // 16, 1)], i16, name=f"idxsb{h}")
                nc.sync.dma_start(out=t[:], in_=idx_d[h][:])
                idx_sb.append(t)
                t2 = constp.tile([128, max(nb_h[h], 1)], f32, name=f"dofsb{h}")
                nc.sync.dma_start(out=t2[:], in_=dof_d[h][:])
                dof_sb.append(t2)
            inval_sb = []
            if S_MODE in ("tt", "ts"):
                for h in range(2):
                    t3 = constp.tile(
                        [128, max(nb_h[h], 1)], f32, name=f"invalsb{h}"
                    )
                    nc.sync.dma_start(out=t3[:], in_=inval_d[h][:])
                    inval_sb.append(t3)
            iota_sb = constp.tile([128, W_N], f16)
            nc.sync.dma_start(out=iota_sb[:], in_=iota_d[:])
            invrow_sb = constp.tile([128, NPAD], f16)
            nc.sync.dma_start(out=invrow_sb[:], in_=invrow_d[:])
            ident = constp.tile([128, 128], f32)
            make_identity(nc, ident[:])
            ident16 = constp.tile([128, 128], f16)
            nc.vector.tensor_copy(ident16[:], ident[:])
            embW_sb = constp.tile([128, KCH, HID], f16)
            nc.sync.dma_start(out=embW_sb[:], in_=embW_d[:].rearrange("k p h -> p k h"))
            embB_sb = constp.tile([128, 1], f32)
            nc.sync.dma_start(out=embB_sb[:], in_=embB_d[:])
            Wl_sb = constp.tile([128, L, HID], f16)
            nc.sync.dma_start(out=Wl_sb[:], in_=Wl_d[:].rearrange("l p h -> p l h"))
            Wr_sb = constp.tile([128, L, HID], f16)
            nc.sync.dma_start(out=Wr_sb[:], in_=Wr_d[:].rearrange("l p h -> p l h"))
            bl_sb = constp.tile([128, L], f32)
            nc.sync.dma_start(out=bl_sb[:], in_=bl_d[:].rearrange("l p one -> p (l one)"))
            hT = [bigp.tile([128, NPAD], f16, name=f"hT{i}") for i in range(2)]
            partA = bigp.tile([128, NPAD], f16, name="partA")  # phase-A means

            # DRAM buffers (fp16 table + per-layer AllGather outputs)
            n_ag = 1 + timing_reps * (L - 1)
            cc_in = [
                dram.tile([NPAD, HID], f16, name=f"ccin{i}", bufs=1) for i in range(2)
            ]
            h_all = [
                [
                    dram.tile(
                        [HALF, HID], f16, name=f"hall{i}_{k}", bufs=1,
                        addr_space="Shared",
                    )
                    for k in range(2)
                ]
                for i in range(n_ag)
            ]

            def writeback(hTbuf, w, dest):
                # transpose window back to node-major (fp16) and DMA to dest rows
                cs = slice(w * W_N, (w + 1) * W_N)
                ptile = pt.tile([128, 128], f16, tag="pt16", name="ptile")
                nc.tensor.transpose(ptile[:], hTbuf[:, cs], ident16[:])
                hsb = hpool.tile([128, 128], f16, tag="hsb", name="hsb")
                nc.scalar.activation(
                    hsb[:], ptile[:], mybir.ActivationFunctionType.Identity
                )
                nc.sync.dma_start(out=dest[cs, :], in_=hsb[:])

            def allgather(src_cc, dst_pair, k):
                ins_ap = src_cc[k * RCH : (k + 1) * RCH, :]
                if _no_ag:
                    nc.sync.dma_start(out=dst_pair[k][0:RCH, :], in_=ins_ap)
                    return
                nc.gpsimd.collective_compute(
                    "AllGather",
                    mybir.AluOpType.bypass,
                    replica_groups=rg,
                    ins=[ins_ap],
                    outs=[dst_pair[k][:]],
                )

            def embedding(xT_sb):
                for w in range(NW):
                    ws = slice(w * W_N, (w + 1) * W_N)
                    pzz = pz.tile([128, W_N], f32, tag="pz", name="pz")
                    for k in range(KCH):
                        nc.tensor.matmul(
                            pzz[:],
                            lhsT=embW_sb[:, k, :],
                            rhs=xT_sb[:, k, ws],
                            start=(k == 0),
                            stop=(k == KCH - 1),
                        )
                    nc.scalar.activation(
                        hT[0][:, ws], pzz[:], mybir.ActivationFunctionType.Relu,
                        bias=embB_sb[:],
                    )
                    writeback(hT[0], w, cc_in[0])
                    if w == NW // 2 - 1:
                        allgather(cc_in[0], h_all[0], 0)
                allgather(cc_in[0], h_all[0], 1)

            def layer(l, h_src, hT_in, hT_out, dest, next_hall, S_all=None):
                half_ap = [h_src[0][:], h_src[1][:]]
                issued = [{}, {}]
                nexti = [0, 0]

                def ensure_chunk(h, ci):
                    while nexti[h] <= ci:
                        w0, n = chunks[h][nexti[h]]
                        nb = n // 128
                        sdt = f8 if S_MODE == "f8" else f16
                        gt = gtp.tile([128, nb, 128], f16, tag="gt", name="gt")
                        if _no_gather:
                            nc.vector.memset(gt[:], 0.0)
                        else:
                            nc.gpsimd.dma_gather(
                                gt[:],
                                half_ap[h],
                                idx_sb[h][:, w0 // 16 : (w0 + n) // 16],
                                n,
                                n,
                                HID,
                                queue_num=next_q(),
                            )
                        if S_MODE == "pf8":
                            # persistent layer-invariant S; no per-chunk build
                            issued[h][nexti[h]] = (gt, None)
                            nexti[h] += 1
                            continue
                        # batched one-hot S for all nb blocks of this chunk,
                        # scaled per-slot by 1/deg(dst) (mean aggregation)
                        b0 = w0 // 128
                        S = sp.tile([128, nb, W_N], sdt, tag="S", name="S")
                        if _no_s:
                            nc.vector.memset(S[:], 0.0)
                        elif S_MODE == "f8":
                            # pure one-hot in fp8 (exact); 1/deg applied to the
                            # flushed partials instead -> single DVE pass.
                            nc.vector.tensor_tensor(
                                out=S[:],
                                in0=dof_sb[h][:, b0 : b0 + nb, None].broadcast_to(
                                    [128, nb, W_N]
                                ),
                                in1=iota_sb[:, None, :].broadcast_to([128, nb, W_N]),
                                op=mybir.AluOpType.is_equal,
                            )
                        elif S_MODE == "dram":
                            # S blocks precomputed on host (pure edge_index
                            # preprocessing); stream from DRAM, no DVE work.
                            nc.sync.dma_start(
                                out=S[:], in_=sblk_d[h][:, b0 : b0 + nb, :]
                            )
                        elif S_MODE == "ts":
                            # fused one-hot build: S[:,j,:] = (iota == dof_j) * inval_j
                            # per-partition scalars keep all tensor operands
                            # packed f16/SBUF -> DVE 4x_2p fast mode.
                            for j in range(nb):
                                nc.vector.tensor_scalar(
                                    out=S[:, j, :],
                                    in0=iota_sb[:],
                                    scalar1=dof_sb[h][:, b0 + j : b0 + j + 1],
                                    scalar2=inval_sb[h][:, b0 + j : b0 + j + 1],
                                    op0=mybir.AluOpType.is_equal,
                                    op1=mybir.AluOpType.mult,
                                )
                        else:
                            nc.vector.tensor_tensor(
                                out=S[:],
                                in0=dof_sb[h][:, b0 : b0 + nb, None].broadcast_to(
                                    [128, nb, W_N]
                                ),
                                in1=iota_sb[:, None, :].broadcast_to([128, nb, W_N]),
                                op=mybir.AluOpType.is_equal,
                            )
                            nc.vector.tensor_tensor(
                                out=S[:],
                                in0=S[:],
                                in1=inval_sb[h][:, b0 : b0 + nb, None].broadcast_to(
                                    [128, nb, W_N]
                                ),
                                op=mybir.AluOpType.mult,
                            )
                        issued[h][nexti[h]] = (gt, S)
                        nexti[h] += 1

                def agg_blocks(h, w, pagg):
                    nb = int(B[h][w])
                    for b in range(nb):
                        ab = int(slot_off[h][w]) + b * 128
                        ci = ab // GMAX
                        ensure_chunk(h, ci)
                        if _ablate:
                            continue
                        gt, S = issued[h][ci]
                        j = (ab % GMAX) // 128
                        rhs = (
                            S_all[h][:, ab // 128, :]
                            if S_MODE == "pf8"
                            else S[:, j, :]
                        )
                        nc.tensor.matmul(
                            pagg[:],
                            lhsT=gt[:, j, :],
                            rhs=rhs,
                            start=(b == 0),
                            stop=(b == nb - 1),
                        )
                    return nb > 0 and not _ablate

                # ---- phase A: half-0 partial means for all windows ----
                if S_MODE in ("f8", "pf8"):
                    # windows processed in pairs sharing one PSUM bank with a
                    # single fused flush+scale per pair: halves the number of
                    # PSUM->SBUF round-trips (sem chains) in phase A.
                    GRP = 4 if NW % 4 == 0 else 2
                    for wp in range(0, NW, GRP):
                        pagg2 = pag.tile(
                            [128, GRP, W_N], f32, tag="pagg", name="pagg"
                        )
                        got = [
                            agg_blocks(0, wp + k, pagg2[:, k, :])
                            for k in range(GRP)
                        ]
                        ws2 = slice(wp * W_N, (wp + GRP) * W_N)
                        if all(got):
                            nc.vector.tensor_tensor(
                                out=partA[:, ws2],
                                in0=pagg2[:].rearrange("p a b -> p (a b)"),
                                in1=invrow_sb[:, ws2],
                                op=mybir.AluOpType.mult,
                            )
                        else:
                            for k in range(GRP):
                                wk = slice(
                                    (wp + k) * W_N, (wp + k + 1) * W_N
                                )
                                if got[k]:
                                    nc.vector.tensor_tensor(
                                        out=partA[:, wk],
                                        in0=pagg2[:, k, :],
                                        in1=invrow_sb[:, wk],
                                        op=mybir.AluOpType.mult,
                                    )
                                else:
                                    nc.vector.memset(partA[:, wk], 0.0)
                else:
                    for w in range(NW):
                        ws = slice(w * W_N, (w + 1) * W_N)
                        pagg = pag.tile([128, W_N], f32, tag="paggs", name="pagg")
                        if agg_blocks(0, w, pagg):
                            nc.scalar.activation(
                                partA[:, ws], pagg[:],
                                mybir.ActivationFunctionType.Identity,
                            )
                        else:
                            nc.vector.memset(partA[:, ws], 0.0)

                # ---- phase B: half-1 + dense + writeback ----
                # the writeback chain (PE transpose -> ACT/DVE copy -> DMA) of
                # window w is issued one window late so the in-order PE queue
                # never stalls on window w's relu before starting w+1's aggs.
                pending = []

                def flush_pending():
                    while pending:
                        pending.pop(0)()

                if S_MODE == "pf8":
                    # strip-dense: phase-B partials are ADDED into partA on the
                    # (idle) DVE, then the dense transform runs in 512-wide
                    # strips: 2 matmuls + 1 relu per strip instead of 3+1 per
                    # 128-window -> ~250 fewer PE insts and 37 fewer ACT insts
                    # per layer.
                    SW = 512
                    for s0 in range(0, NPAD, SW):
                        sw = min(SW, NPAD - s0)
                        for w in range(s0 // W_N, (s0 + sw) // W_N):
                            ws = slice(w * W_N, (w + 1) * W_N)
                            pagg = pag.tile([128, W_N], f32, tag="paggs", name="pagg")
                            if agg_blocks(1, w, pagg):
                                tmp = apool.tile([128, W_N], f16, tag="p2", name="p2")
                                nc.vector.tensor_tensor(
                                    out=tmp[:], in0=pagg[:],
                                    in1=invrow_sb[:, ws], op=mybir.AluOpType.mult,
                                )
                                nc.vector.tensor_tensor(
                                    out=partA[:, ws], in0=partA[:, ws],
                                    in1=tmp[:], op=mybir.AluOpType.add,
                                )
                        while len(pending) > 1:
                            pending.pop(0)()
                        ss = slice(s0, s0 + sw)
                        pzz = pz.tile([128, SW], f32, tag="pz", name="pz")
                        nc.tensor.matmul(
                            pzz[:, :sw], lhsT=Wl_sb[:, l, :], rhs=partA[:, ss],
                            start=True, stop=False,
                        )
                        nc.tensor.matmul(
                            pzz[:, :sw], lhsT=Wr_sb[:, l, :], rhs=hT_in[:, ss],
                            start=False, stop=True,
                        )
                        if l < L - 1:
                            nc.scalar.activation(
                                hT_out[:, ss], pzz[:, :sw],
                                mybir.ActivationFunctionType.Relu,
                                bias=bl_sb[:, l : l + 1],
                            )

                            def wb(s0=s0, sw=sw):
                                for w in range(s0 // W_N, (s0 + sw) // W_N):
                                    writeback(hT_out, w, dest)
                                    if w == NW // 2 - 1:
                                        allgather(dest, next_hall, 0)
                                    elif w == NW - 1:
                                        allgather(dest, next_hall, 1)

                            pending.append(wb)
                        else:
                            h4s = apool.tile([128, SW], f16, tag="h4s", name="h4s")
                            nc.scalar.activation(
                                h4s[:, :sw], pzz[:, :sw],
                                mybir.ActivationFunctionType.Relu,
                                bias=bl_sb[:, l : l + 1],
                            )

                            def wb(s0=s0, sw=sw, h4s=h4s):
                                for wi, w in enumerate(
                                    range(s0 // W_N, (s0 + sw) // W_N)
                                ):
                                    cs = slice(w * W_N, (w + 1) * W_N)
                                    ptile = pt.tile(
                                        [128, 128], f16, tag="pt16", name="ptile"
                                    )
                                    nc.tensor.transpose(
                                        ptile[:],
                                        h4s[:, wi * 128 : (wi + 1) * 128],
                                        ident16[:],
                                    )
                                    hsb = hpool.tile(
                                        [128, 128], f32, tag="hsbo", name="hsbo"
                                    )
                                    nc.vector.tensor_copy(hsb[:], ptile[:])
                                    nc.sync.dma_start(out=out_d[cs, :], in_=hsb[:])

                            pending.append(wb)
                    flush_pending()
                    return

                def dense_relu_wb(w, got, part2_ap):
                    ws = slice(w * W_N, (w + 1) * W_N)
                    pzz = pz.tile([128, W_N], f32, tag="pz", name="pz")
                    nc.tensor.matmul(
                        pzz[:], lhsT=Wl_sb[:, l, :], rhs=partA[:, ws],
                        start=True, stop=False,
                    )
                    if got:
                        nc.tensor.matmul(
                            pzz[:], lhsT=Wl_sb[:, l, :], rhs=part2_ap,
                            start=False, stop=False,
                        )
                    nc.tensor.matmul(
                        pzz[:], lhsT=Wr_sb[:, l, :], rhs=hT_in[:, ws], start=False,
                        stop=True,
                    )
                    if l < L - 1:
                        nc.scalar.activation(
                            hT_out[:, ws], pzz[:],
                            mybir.ActivationFunctionType.Relu,
                            bias=bl_sb[:, l : l + 1],
                        )

                        def wb(w=w):
                            writeback(hT_out, w, dest)
                            if w == NW // 2 - 1:
                                allgather(dest, next_hall, 0)
                            elif w == NW - 1:
                                allgather(dest, next_hall, 1)

                        pending.append(wb)
                    else:
                        h4 = apool.tile([128, W_N], f16, tag="h4", name="h4")
                        nc.scalar.activation(
                            h4[:], pzz[:],
                            mybir.ActivationFunctionType.Relu,
                            bias=bl_sb[:, l : l + 1],
                        )

                        def wb(w=w, h4=h4):
                            cs = slice(w * W_N, (w + 1) * W_N)
                            ptile = pt.tile([128, 128], f16, tag="pt16", name="ptile")
                            nc.tensor.transpose(ptile[:], h4[:], ident16[:])
                            hsb = hpool.tile([128, 128], f32, tag="hsbo", name="hsbo")
                            nc.vector.tensor_copy(hsb[:], ptile[:])
                            nc.sync.dma_start(out=out_d[cs, :], in_=hsb[:])

                        pending.append(wb)
                    if len(pending) > 1:
                        pending.pop(0)()

                if S_MODE in ("f8", "pf8"):
                    # paired phase-B: two windows share one PSUM bank and one
                    # fused flush+scale, then each window's dense runs.
                    GRP = 4 if NW % 4 == 0 else 2
                    for wp in range(0, NW, GRP):
                        pagg2b = pag.tile(
                            [128, GRP, W_N], f32, tag="pagg", name="pagg"
                        )
                        gots = [
                            agg_blocks(1, wp + k, pagg2b[:, k, :])
                            for k in range(GRP)
                        ]
                        part2 = apool.tile(
                            [128, GRP, W_N], f16, tag="p2", name="p2"
                        )
                        ws2 = slice(wp * W_N, (wp + GRP) * W_N)
                        if all(gots):
                            nc.vector.tensor_tensor(
                                out=part2[:].rearrange("p a b -> p (a b)"),
                                in0=pagg2b[:].rearrange("p a b -> p (a b)"),
                                in1=invrow_sb[:, ws2],
                                op=mybir.AluOpType.mult,
                            )
                        else:
                            for k in range(GRP):
                                if gots[k]:
                                    wk = slice(
                                        (wp + k) * W_N, (wp + k + 1) * W_N
                                    )
                                    nc.vector.tensor_tensor(
                                        out=part2[:, k, :],
                                        in0=pagg2b[:, k, :],
                                        in1=invrow_sb[:, wk],
                                        op=mybir.AluOpType.mult,
                                    )
                        for k in range(GRP):
                            dense_relu_wb(wp + k, gots[k], part2[:, k, :])
                    flush_pending()
                    return

                for w in range(NW):
                    ws = slice(w * W_N, (w + 1) * W_N)
                    pagg = pag.tile([128, W_N], f32, tag="paggs", name="pagg")
                    got = agg_blocks(1, w, pagg)
                    if got:
                        part2 = apool.tile([128, W_N], f16, tag="p2s", name="p2")
                        nc.scalar.activation(
                            part2[:], pagg[:],
                            mybir.ActivationFunctionType.Identity,
                        )
                    pzz = pz.tile([128, W_N], f32, tag="pz", name="pz")
                    nc.tensor.matmul(
                        pzz[:], lhsT=Wl_sb[:, l, :], rhs=partA[:, ws],
                        start=True, stop=False,
                    )
                    if got:
                        nc.tensor.matmul(
                            pzz[:], lhsT=Wl_sb[:, l, :], rhs=part2[:],
                            start=False, stop=False,
                        )
                    nc.tensor.matmul(
                        pzz[:], lhsT=Wr_sb[:, l, :], rhs=hT_in[:, ws], start=False,
                        stop=True,
                    )
                    if l < L - 1:
                        nc.scalar.activation(
                            hT_out[:, ws], pzz[:],
                            mybir.ActivationFunctionType.Relu,
                            bias=bl_sb[:, l : l + 1],
                        )

                        def wb(w=w):
                            writeback(hT_out, w, dest)
                            if w == NW // 2 - 1:
                                allgather(dest, next_hall, 0)
                            elif w == NW - 1:
                                allgather(dest, next_hall, 1)

                        pending.append(wb)
                    else:
                        h4 = apool.tile([128, W_N], f16, tag="h4", name="h4")
                        nc.scalar.activation(
                            h4[:], pzz[:],
                            mybir.ActivationFunctionType.Relu,
                            bias=bl_sb[:, l : l + 1],
                        )

                        def wb(w=w, h4=h4):
                            cs = slice(w * W_N, (w + 1) * W_N)
                            ptile = pt.tile([128, 128], f16, tag="pt16", name="ptile")
                            nc.tensor.transpose(ptile[:], h4[:], ident16[:])
                            hsb = hpool.tile([128, 128], f32, tag="hsbo", name="hsbo")
                            nc.vector.tensor_copy(hsb[:], ptile[:])
                            nc.sync.dma_start(out=out_d[cs, :], in_=hsb[:])

                        pending.append(wb)
                    if len(pending) > 1:
                        pending.pop(0)()
                flush_pending()

            import os as _os2
            _noscope = _os2.environ.get("KERNEL_PF8_NOSCOPE", "0") == "1"
            if _noscope:
                xT_sb = bigp.tile([128, KCH, NPAD], f16, name="xT")
                nc.sync.dma_start(
                    out=xT_sb[:], in_=xT_d[:].rearrange("k p n -> p k n")
                )
                embedding(xT_sb)
            else:
                with tc.tile_pool(name="xt", bufs=1) as xtp:
                    xT_sb = xtp.tile([128, KCH, NPAD], f16)
                    nc.sync.dma_start(
                        out=xT_sb[:], in_=xT_d[:].rearrange("k p n -> p k n")
                    )
                    embedding(xT_sb)

            def run_layers(S_all=None):
                agi = 0
                for rep in range(timing_reps):
                    for l in range(L):
                        layer(
                            l,
                            h_all[agi],
                            hT[l % 2],
                            hT[(l + 1) % 2],
                            cc_in[(l + 1) % 2],
                            h_all[agi + 1] if l < L - 1 else None,
                            S_all=S_all,
                        )
                        if l < L - 1:
                            agi += 1

            if S_MODE == "pf8":
                # layer-invariant one-hot S built once in fp8, resident in the
                # SBUF region vacated by the embedding's xT tile
                with tc.tile_pool(name="sall", bufs=1) as sallp:
                    S_all = [
                        sallp.tile(
                            [128, max(nb_h[h], 1), W_N], f8, name=f"sall{h}"
                        )
                        for h in range(2)
                    ]
                    for h in range(2):
                        for w0, n in chunks[h]:
                            nb = n // 128
                            b0 = w0 // 128
                            nc.vector.tensor_tensor(
                                out=S_all[h][:, b0 : b0 + nb, :],
                                in0=dof_sb[h][:, b0 : b0 + nb, None].broadcast_to(
                                    [128, nb, W_N]
                                ),
                                in1=iota_sb[:, None, :].broadcast_to(
                                    [128, nb, W_N]
                                ),
                                op=mybir.AluOpType.is_equal,
                            )
                    run_layers(S_all)
            else:
                run_layers()

    nc.compile()
    return nc


def _prep_inputs(inputs, struct):
    x = np.asarray(inputs["x"], dtype=np.float32)
    emb_W = np.asarray(inputs["emb_W"], dtype=np.float32)
    emb_b = np.asarray(inputs["emb_b"], dtype=np.float32)
    Wl = np.asarray(inputs["Wl"], dtype=np.float32)
    bl = np.asarray(inputs["bl"], dtype=np.float32)
    Wr = np.asarray(inputs["Wr"], dtype=np.float32)

    embW_p = np.zeros((KCH, 128, HID), dtype=np.float16)
    embW_p.reshape(KCH * 128, HID)[:IN_DIM] = emb_W.astype(np.float16)
    embB_p = np.zeros((128, 1), dtype=np.float32)
    embB_p[:, 0] = emb_b
    Wl_p = Wl.astype(np.float16)
    Wr_p = Wr.astype(np.float16)
    bl_p = np.ascontiguousarray(bl[:, :, None].astype(np.float32))

    iota = np.broadcast_to(
        np.arange(W_N, dtype=np.float16)[None, :], (128, W_N)
    ).copy()

    in_maps = []
    for m in range(NC):
        xm = np.zeros((KCH * 128, NPAD), dtype=np.float16)
        xm[:IN_DIM, :NPC] = x[m * NPC : (m + 1) * NPC].T.astype(np.float16)
        im = {
            "idx0": struct["idx_wrapped"][m][0],
            "idx1": struct["idx_wrapped"][m][1],
            "dof0": struct["dof"][m][0],
            "dof1": struct["dof"][m][1],
            "inval0": struct["invslot"][m][0],
            "inval1": struct["invslot"][m][1],
            **(
                {"sblk0": struct["sblk"][m][0], "sblk1": struct["sblk"][m][1]}
                if S_DRAM
                else {}
            ),
            "xT": xm.reshape(KCH, 128, NPAD),
            "embW": embW_p,
            "embB": embB_p,
            "Wl": Wl_p,
            "Wr": Wr_p,
            "bl": bl_p,
            "iota": iota,
            "invrow": struct["invrow"][m],
        }
        in_maps.append(im)
    return in_maps


class BassRunner:
    """Executes a compiled Bass program via PJRT/axon; jit built once."""

    def __init__(self, nc, n_cores):
        import jax
        from jax.sharding import Mesh, PartitionSpec
        from jax.experimental.shard_map import shard_map
        from concourse.bass2jax import (
            _bass_exec_p,
            install_neuronx_cc_hook,
            partition_id_tensor,
        )

        install_neuronx_cc_hook()
        self.jax = jax
        self.nc = nc
        self.n_cores = n_cores
        partition_name = (
            nc.partition_id_tensor.name if nc.partition_id_tensor else None
        )
        in_names, out_names, out_avals, zero_outs = [], [], [], []
        for alloc in nc.m.functions[0].allocations:
            if not isinstance(alloc, mybir.MemoryLocationSet):
                continue
            name = alloc.memorylocations[0].name
            if alloc.kind == "ExternalInput":
                if name != partition_name:
                    in_names.append(name)
            elif alloc.kind == "ExternalOutput":
                shape = tuple(alloc.tensor_shape)
                dtype = mybir.dt.np(alloc.dtype)
                out_names.append(name)
                out_avals.append(jax.core.ShapedArray(shape, dtype))
                zero_outs.append(np.zeros(shape, dtype))
        self.in_names, self.out_names = in_names, out_names
        self.zero_outs, self._out_avals = zero_outs, out_avals
        n_params, n_outs = len(in_names), len(out_avals)
        all_in_names = in_names + out_names
        if partition_name is not None:
            all_in_names = all_in_names + [partition_name]

        def _body(*args):
            operands = list(args)
            if partition_name is not None:
                operands.append(partition_id_tensor())
            return tuple(
                _bass_exec_p.bind(
                    *operands,
                    out_avals=tuple(out_avals),
                    in_names=tuple(all_in_names),
                    out_names=tuple(out_names),
                    lowering_input_output_aliases=(),
                    sim_require_finite=True,
                    sim_require_nnan=True,
                    nc=nc,
                )
            )

        devices = jax.devices()[:n_cores]
        self._mesh = Mesh(np.asarray(devices), ("core",))
        self._pspec = PartitionSpec("core")
        in_specs = (self._pspec,) * (n_params + n_outs)
        out_specs = (self._pspec,) * len(out_names)
        self._fn = jax.jit(
            shard_map(
                _body,
                mesh=self._mesh,
                in_specs=in_specs,
                out_specs=out_specs,
                check_rep=False,
            ),
            keep_unused=True,
        )

    def prepare(self, in_maps):
        n = self.n_cores
        concat_in = [
            np.concatenate(
                [np.asarray(in_maps[c][name]) for c in range(n)], axis=0
            )
            for name in self.in_names
        ]
        concat_zeros = [
            np.zeros((n * z.shape[0], *z.shape[1:]), z.dtype)
            for z in self.zero_outs
        ]
        sharding = self.jax.sharding.NamedSharding(self._mesh, self._pspec)
        self._args = [
            self.jax.device_put(a, sharding) for a in concat_in + concat_zeros
        ]

    def execute(self):
        outs = self._fn(*self._args)
        self.jax.block_until_ready(outs)
        return outs

    def run(self):
        outs = self.execute()
        n = self.n_cores
        return [
            {
                name: np.asarray(outs[i]).reshape(
                    n, *self._out_avals[i].shape
                )[c]
                for i, name in enumerate(self.out_names)
            }
            for c in range(n)
        ]


def _get_runner(edge_index, timing_reps=1):
    import os as _os
    _flags = (
        _os.environ.get("KERNEL_NO_AG", ""),
        _os.environ.get("KERNEL_ABLATE", ""),
        _os.environ.get("KERNEL_NO_GATHER", ""),
        _os.environ.get("KERNEL_NO_S", ""),
        GMAX,
        DMA_SCRATCH,
        S_MODE,
        PAG_BUFS,
        GT_BUFS,
        NQ,
        PT_BUFS,
    )
    key = ("prog", timing_reps, _flags, hash(edge_index.tobytes()))
    if key in _CACHE:
        return _CACHE[key]
    struct = _host_prep(edge_index)
    nc = _build_program(struct, timing_reps=timing_reps)
    runner = BassRunner(nc, NC)
    _CACHE[key] = (struct, runner)
    return struct, runner


def kernel(**inputs):
    edge_index = np.asarray(inputs["edge_index"])
    struct, runner = _get_runner(edge_index)
    in_maps = _prep_inputs(inputs, struct)
    runner.prepare(in_maps)
    results = runner.run()
    out = np.empty((N, HID), dtype=np.float32)
    for m in range(NC):
        out[m * NPC : (m + 1) * NPC] = results[m]["out"][:NPC]
    return out

